# revision 14
# baseline (speedup 1.0000x reference)
"""Multi-head self-attention (B=4, S=2048, D=1024, H=16) on 8 trn2 NeuronCores.

Sharding: batch (4) x head-group (2 groups of 8 heads) -> 8 cores.
Each core computes, for its (batch b, head-group hg):
  Q'^T = (wq_l/8) @ x_b^T            [512, 2048]   (1/sqrt(dk) folded into wq)
  K^T  = wk_l @ x_b^T                [512, 2048]
  V    = x_b @ wv_l^T                [2048, 512]
  per head h (8 local, dk=64), in transposed layout (keys on partitions):
    scoresT[k, q] = K_h @ Q'_h^T     (no max-subtraction: scores ~ N(0,4), exp
                                      of |s|<~12 is safe in fp32/bf16)
    expT = exp(scoresT)              (ScalarE, PSUM->SBUF bf16)
    unnormT[c, q] = V_h^T @ expT     (PE, accumulated over key tiles)
    Z[q] = ones^T @ expT             (PE colsum quads, same accumulation)
    attnT = unnormT / Z              (reciprocal once + DMA partition
                                      broadcast via DRAM + DVE muls)
  out_partial = attnT^T @ wo_l^T     [2048, 1024]  (row-parallel wo)
Host sums the two partials per batch (the "all-reduce" of row-parallel wo).

v2 schedule: the 256 exps (ScalarE, ~1.11us each) are the pacer.  Blocks run
qh-outer (all 4 head-pairs for queries 0-1023, then 1024-2047) so the output
projection for the first query half can overlap the second attention phase.
All projection work except the first three Q/K chunks is deferred into the
attention stream as fine-grained (2-matmul) filler pieces pulled by a
deadline-driven queue, so the PE never blocks the exp chain for more than
~0.5us at a time.  DMA is staged (wk+xq0 first, then wq+wv, then the rest) so
the first scores matmul can issue at ~15us.
"""

import ml_dtypes
import numpy as np

import bass_rust
import concourse.bass as bass
import concourse.mybir as mybir
import concourse.tile as tile

# ---------------------------------------------------------------- constants
S = 2048          # sequence length
DM = 1024         # model dim
DL = 512          # local (per-core) head dims = 8 heads * 64
DK = 64           # head dim
P = 128
NKT = S // P      # 16 key tiles
NG = DL // P      # 4 head-pairs (c-tiles / dq-tiles)
KD = DM // P      # 8 contraction tiles for projections
NSC = S // 512    # 4 s-chunks for projections
F32 = mybir.dt.float32
BF16 = mybir.dt.bfloat16
BF16_NP = ml_dtypes.bfloat16

N_CORES = 8
CORE_IDS = list(range(N_CORES))


# ------------------------------------------------- walrus sync-wait workaround
def _split_sync_waits(nc, limit=1):
    """This toolchain's walrus codegen rejects instructions carrying more than
    one sync-wait command.  Move excess waits onto dedicated same-engine nops
    inserted immediately before the instruction (sequential waits on the same
    engine queue are semantically identical to multiple waits on one inst)."""
    fn = nc.m.functions[0]
    snapshots = [(bb, list(bb.instructions)) for bb in fn.blocks]
    plans = []
    for _bb, insts in snapshots:
        plan = {}
        for idx, inst in enumerate(insts):
            si = inst.sync_info
            waits = list(si.on_wait) if si and si.on_wait else []
            if len(waits) > limit:
                pre, keep = waits[:-limit], waits[-limit:]
                nops = []
                for w in pre:
                    ni = nc.engines[inst.engine].nop(nofuse=True, hint="wsplit").ins
                    ni.sync_info = bass_rust.SyncInfo(on_wait=[w], on_update=[])
                    nops.append(ni)
                si.on_wait = keep
                plan[idx] = nops
        plans.append(plan)
    # Rebuild every block from its pre-pass snapshot plus insertions; this also
    # drops the fresh nops from wherever bass appended them at creation time.
    for (bb, insts), plan in zip(snapshots, plans):
        out = []
        for idx, inst in enumerate(insts):
            out.extend(plan.get(idx, ()))
            out.append(inst)
        bb.instructions = out


# ---------------------------------------------------------------- the program
def build_nc():
    """Build the SPMD per-core Bass program (identical on all 8 cores)."""
    nc = bass.Bass()

    xT = nc.declare_dram_parameter("xT", [DM, S], BF16, isOutput=False)
    wqT = nc.declare_dram_parameter("wqT", [DM, DL], BF16, isOutput=False)
    wkT = nc.declare_dram_parameter("wkT", [DM, DL], BF16, isOutput=False)
    wvT = nc.declare_dram_parameter("wvT", [DM, DL], BF16, isOutput=False)
    woT = nc.declare_dram_parameter("woT", [DL, DM], BF16, isOutput=False)
    out = nc.declare_dram_parameter("out", [S, DM], F32, isOutput=True)

    with tile.TileContext(nc) as tc:
        with (
            tc.tile_pool(name="big", bufs=1) as big,
            tc.tile_pool(name="expT", bufs=16) as expp,
            tc.tile_pool(name="rc", bufs=2) as rcp,
            tc.tile_pool(name="outsb", bufs=3) as outp,
            tc.tile_pool(name="zacc", bufs=2) as zap,
            tc.tile_pool(name="dram", bufs=2, space="DRAM") as dramp,
            tc.tile_pool(name="ps", bufs=2, space="PSUM") as psp,
            tc.tile_pool(name="acc", bufs=1, space="PSUM") as accp,
            tc.tile_pool(name="fil", bufs=1, space="PSUM") as filp,
        ):
            # ---------------- staged DRAM loads.  Stage 1 (wk + first x
            # quarter) ships alone so the first K/Q projection chunks can
            # start at ~15us; later stages are gated behind earlier tensors
            # via 1-element gpsimd copies (RAW on the gating tile, WAW on the
            # staged destination) so they don't steal HBM bandwidth early.
            w_sb = {}
            for name in ("wk", "wq", "wv"):
                w_sb[name] = big.tile([P, KD, DL], BF16, tag=name, name=name)
            xT_r = xT.rearrange("(kd p) s -> p kd s", p=P)
            xT_q = [
                big.tile([P, KD, 512], BF16, tag=f"xT{j}", name=f"xTq{j}")
                for j in range(4)
            ]
            woT_sb = big.tile([P, NG, DM], BF16, tag="wo")

            def stage(dsts, gate):
                if gate is not None:
                    for d in dsts:
                        nc.gpsimd.tensor_copy(
                            out=d[0:1, 0, 0:1], in_=gate[0:1, 0, 0:1]
                        )
            # stage 1: wk, wq, xq0 (feeds the upfront K0/Q0 chunks)
            stage([w_sb["wk"], w_sb["wq"], xT_q[0]], None)
            nc.sync.dma_start(
                w_sb["wk"][:], wkT.rearrange("(kd p) m -> p kd m", p=P)
            )
            nc.sync.dma_start(
                w_sb["wq"][:], wqT.rearrange("(kd p) m -> p kd m", p=P)
            )
            nc.sync.dma_start(xT_q[0][:], xT_r[:, :, 0:512])
            # stage 2: wv + xq1 (gated on wk): Q0 sc1 reads x cols 512-1023
            stage([w_sb["wv"], xT_q[1]], w_sb["wk"])
            nc.sync.dma_start(
                w_sb["wv"][:], wvT.rearrange("(kd p) m -> p kd m", p=P)
            )
            nc.sync.dma_start(xT_q[1][:], xT_r[:, :, 512:1024])
            # stage 3: xq2, then xq3+wo chained behind
            stage([xT_q[2]], xT_q[1])
            nc.sync.dma_start(xT_q[2][:], xT_r[:, :, 1024:1536])
            stage([xT_q[3], woT_sb], xT_q[2])
            nc.sync.dma_start(xT_q[3][:], xT_r[:, :, 1536:2048])
            nc.sync.dma_start(woT_sb[:], woT.rearrange("(ct p) o -> p ct o", p=P))

            def xslice(kd, fr, to):
                q = fr // 512
                assert to <= (q + 1) * 512
                return xT_q[q][:, kd, fr - q * 512 : to - q * 512]

            # ---------------- constants
            ones_bf = big.tile([P, 1], BF16, tag="ones")
            nc.vector.memset(ones_bf[:], 1.0)
            warm = big.tile([P, 512], BF16, tag="warm")
            nc.vector.memset(warm[:], 0.0)

            # HAM warm-up: keep the PE busy on throwaway matmuls while the
            # stage-1 DMA lands so the first real chunks run at 2.4GHz.
            ps_w = psp.tile([P, 512], F32, tag="ps", name="warmps")
            for _ in range(24):
                nc.tensor.matmul(
                    ps_w[:], lhsT=warm[:, 0:128], rhs=warm[:], start=True,
                    stop=True,
                )

            # persistent activation tensors
            QT = [big.tile([P, S], BF16, tag=f"QT{g}", name=f"QT{g}") for g in range(NG)]
            KT = [big.tile([P, S], BF16, tag=f"KT{g}", name=f"KT{g}") for g in range(NG)]
            V_st = [big.tile([P, 8, DK + 1], BF16, tag=f"V{st}", name=f"V{st}") for st in range(NKT)]
            attn = [big.tile([P, S], BF16, tag=f"attn{g}", name=f"attn{g}") for g in range(NG)]

            # ---------------- filler machinery: all projection / output work
            # is expressed as jobs that yield 2-matmul pieces; the attention
            # driver pulls pieces by deadline so the exp chain never waits
            # long on the PE queue.
            def qk_job(dst, w, g, sc):
                ps = filp.tile([P, 512], F32, tag="fil", name="projch")
                for kd0 in range(0, KD, 2):
                    for kd in (kd0, kd0 + 1):
                        nc.tensor.matmul(
                            ps[:],
                            lhsT=w[:, kd, g * P : (g + 1) * P],
                            rhs=xslice(kd, sc * 512, (sc + 1) * 512),
                            start=(kd == 0),
                            stop=(kd == KD - 1),
                        )
                    yield
                nc.vector.tensor_copy(
                    out=dst[:, sc * 512 : (sc + 1) * 512], in_=ps[:]
                )

            def v_job(st):
                ps = filp.tile([P, 512], F32, tag="fil", name="vch")
                for kd0 in range(0, KD, 2):
                    for kd in (kd0, kd0 + 1):
                        nc.tensor.matmul(
                            ps[:],
                            lhsT=xslice(kd, st * P, (st + 1) * P),
                            rhs=w_sb["wv"][:, kd, :],
                            start=(kd == 0),
                            stop=(kd == KD - 1),
                        )
                    yield
                nc.vector.tensor_copy(
                    out=V_st[st][:, :, 0:DK],
                    in_=ps.rearrange("p (h c) -> p h c", c=DK),
                )

            def wo_job(st, ob):
                ps = filp.tile([P, 512], F32, tag="fil", name="wochunk")
                for ct0 in (0, 2):
                    for ct in (ct0, ct0 + 1):
                        nc.tensor.matmul(
                            ps[:],
                            lhsT=attn[ct][:, st * P : (st + 1) * P],
                            rhs=woT_sb[:, ct, ob * 512 : (ob + 1) * 512],
                            start=(ct == 0),
                            stop=(ct == NG - 1),
                        )
                    yield
                ot = outp.tile([P, 512], F32, tag="out")
                nc.vector.tensor_copy(out=ot[:], in_=ps[:])
                nc.sync.dma_start(
                    out[st * P : (st + 1) * P, ob * 512 : (ob + 1) * 512], ot[:]
                )

            class Filler:
                """Ordered queue of (deadline_tick, ready_tick, job-generator).
                One job open at a time (so filler holds a single av psum
                slot); pieces are pulled per tick: everything past deadline
                unconditionally, plus up to `budget` opportunistic pieces."""

                def __init__(self):
                    self.jobs = []
                    self.open = None
                    self.open_deadline = 0

                def add(self, deadline, ready, gen):
                    self.jobs.append((deadline, ready, gen))

                def _pull_one(self, tick):
                    if self.open is None:
                        if not self.jobs or self.jobs[0][1] > tick:
                            return False
                        self.open_deadline, _, self.open = self.jobs.pop(0)
                    try:
                        next(self.open)
                    except StopIteration:
                        self.open = None
                    return True

                def tick(self, tick, budget=2):
                    n = 0
                    while True:
                        urgent = (
                            self.open is not None and self.open_deadline <= tick + 1
                        ) or (
                            self.open is None
                            and self.jobs
                            and self.jobs[0][0] <= tick + 1
                            and self.jobs[0][1] <= tick
                        )
                        if not urgent and n >= budget:
                            break
                        if not self._pull_one(tick):
                            break
                        n += 1

                def drain(self):
                    while self._pull_one(1 << 30):
                        pass

            filler = Filler()

            # ---------------- attention
            class AttnBlock:
                """Heads A=2g, B=2g+1; query half qh (1024 queries).

                scoresT/exp are ACT-paced.  V matmuls run a few kt behind
                (pending FIFO in the driver) so both heads' exp tiles are
                ready together, letting adjacently issued matmuls with
                disjoint array col groups (V: 0-1 vs 2-3) run concurrently
                on the PE.  vt accumulates A in partitions 0-63 and B in
                64-127 of one bank (memset + start=False keeps the
                interleaved accumulation groups from clearing each other).
                The softmax denominators are accumulated OFF the PE: per-kt
                elementwise adds of the exp tiles into a [128,1024] bf16
                accumulator (GpSimd for head A, DVE for head B), reduced
                across partitions by a single 4-matmul quad at block end.
                Normalization runs entirely off the critical path.
                """

                def __init__(self, g, qh):
                    self.g, self.qoff = g, qh * 1024
                    self.vt = [
                        accp.tile([P, 512], F32, tag=f"vt{qb}", name=f"vt{qb}")
                        for qb in range(2)
                    ]
                    for t in self.vt:
                        nc.vector.memset(t[:], 0.0)
                    self.za = [
                        zap.tile([P, 1024], BF16, tag=f"za{hp}", name=f"za{hp}")
                        for hp in range(2)
                    ]
                    nc.gpsimd.memset(self.za[0][:], 0.0)
                    nc.vector.memset(self.za[1][:], 0.0)
                    self.ets = {}

                def emit_scores_exp(self, kt):
                    g, qoff = self.g, self.qoff
                    for hp, pb in ((0, 0), (1, 64)):
                        ps_s = psp.tile([P, 1024], F32, tag="ps", name=f"ps_s{hp}")
                        for qb in range(2):
                            nc.tensor.matmul(
                                ps_s[:, qb * 512 : (qb + 1) * 512],
                                lhsT=KT[g][pb : pb + 64, kt * P : (kt + 1) * P],
                                rhs=QT[g][
                                    pb : pb + 64,
                                    qoff + qb * 512 : qoff + (qb + 1) * 512,
                                ],
                                start=True,
                                stop=True,
                            )
                        et = expp.tile([P, 1024], BF16, tag="expT", name=f"et{hp}")
                        nc.scalar.activation(
                            et[:], ps_s[:], mybir.ActivationFunctionType.Exp
                        )
                        eng = nc.gpsimd if hp == 0 else nc.vector
                        eng.tensor_tensor(
                            out=self.za[hp][:],
                            in0=self.za[hp][:],
                            in1=et[:],
                            op=mybir.AluOpType.add,
                        )
                        self.ets[(kt, hp)] = et

                def emit_v_cs(self, kt):
                    g = self.g
                    last = kt == NKT - 1
                    et = {hp: self.ets.pop((kt, hp)) for hp in (0, 1)}
                    for qb in range(2):
                        for hp, pb in ((0, 0), (1, 64)):
                            nc.tensor.matmul(
                                self.vt[qb][pb : pb + 64, :],
                                lhsT=V_st[kt][:, 2 * g + hp, 0:DK],
                                rhs=et[hp][:, qb * 512 : (qb + 1) * 512],
                                start=False,
                                stop=last,
                                skip_group_check=True,
                                tile_position=(0, pb),
                            )
                    if last:
                        self.emit_norm()

                def emit_norm(self):
                    g, qoff = self.g, self.qoff
                    un = [
                        rcp.tile([P, 512], F32, tag=f"un{qb}", name=f"un{qb}")
                        for qb in range(2)
                    ]
                    for qb in range(2):
                        nc.vector.tensor_copy(out=un[qb][:], in_=self.vt[qb][:])
                    # Z = colsum(exp): partition-reduce the za accumulators
                    # with a concurrent 4-matmul quad into cs rows 0/32/64/96
                    cs = accp.tile([P, 512], F32, tag="cs")
                    for hp in (0, 1):
                        for qb in range(2):
                            cp = 64 * hp + 32 * qb
                            nc.tensor.matmul(
                                cs[cp : cp + 1, :],
                                lhsT=ones_bf[:],
                                rhs=self.za[hp][:, qb * 512 : (qb + 1) * 512],
                                start=True,
                                stop=True,
                                skip_group_check=True,
                                tile_position=(0, cp),
                            )
                    cs_sb = rcp.tile([P, 512], F32, tag="cs_sb")
                    nc.vector.tensor_copy(out=cs_sb[:], in_=cs[:])
                    zd = dramp.tile([4, 512], F32, name="zd")
                    # zd rows: 0=(A,qb0) 1=(A,qb1) 2=(B,qb0) 3=(B,qb1)
                    nc.sync.dma_start(zd[:], cs_sb[0:128:32, :])
                    # reciprocal on a [128,16] reshape of the 2048 real Z
                    # values (vs [128,512]: DVE reciprocal is ~8 cyc/col)
                    zs = rcp.tile([P, 16], F32, tag="zs")
                    nc.sync.dma_start(
                        zs[:], zd.rearrange("a (b c) -> (a b) c", c=16)
                    )
                    zr = rcp.tile([P, 16], F32, tag="zr")
                    nc.vector.reciprocal(zr[:], zs[:])
                    zd2 = dramp.tile([4, 512], F32, name="zd2")
                    nc.sync.dma_start(
                        zd2.rearrange("a (b c) -> (a b) c", c=16), zr[:]
                    )
                    for qb in range(2):
                        rcb = rcp.tile(
                            [P, 512], F32, tag=f"rcb{qb}", name=f"rcb{qb}"
                        )
                        nc.sync.dma_start(
                            rcb[0:64, :], zd2[qb, None, :].to_broadcast([64, 512])
                        )
                        nc.sync.dma_start(
                            rcb[64:128, :],
                            zd2[qb + 2, None, :].to_broadcast([64, 512]),
                        )
                        for pb in (0, 64):
                            nc.vector.tensor_mul(
                                out=attn[g][
                                    pb : pb + 64,
                                    qoff + qb * 512 : qoff + (qb + 1) * 512,
                                ],
                                in0=un[qb][pb : pb + 64, :],
                                in1=rcb[pb : pb + 64, :],
                            )

            # ---------------- upfront projections: just enough for the first
            # scores matmul (K0 cols 0-511, Q0 cols 0-1023)
            for job in (
                qk_job(KT[0], w_sb["wk"], 0, 0),
                qk_job(QT[0], w_sb["wq"], 0, 0),
                qk_job(QT[0], w_sb["wq"], 0, 1),
            ):
                for _ in job:
                    pass

            # ---------------- filler job schedule (deadlines in global ticks;
            # 1 tick = one (block, kt) step = ~2.2us of exp pacing).
            # Block order: qh-outer -> block index B = qh*4 + g.
            # V_st[st] feeds emit_v_cs at tick st + LAG of block 0.
            # K[g][sc] feeds scores of block g (qh0) at kt=4*sc.
            # Q[g][qh sc] feeds block qh*4+g from its start.
            jobs = []
            # V_st[st] is consumed by the block-0 v_cs pop at pop_tick[st];
            # derive that from the same lag schedule the driver uses so the
            # projection is always fully emitted before its consumer.
            pop_tick = {}
            sim_pending, t = [], 0
            while len(pop_tick) < NKT:
                lag = 7 if t < 8 else max(1, 14 - t)
                while len(sim_pending) > lag:
                    pop_tick[sim_pending.pop(0)] = t
                sim_pending.append(t)
                t += 1
            for st in range(NKT):
                jobs.append((pop_tick[st] - 1, max(0, st // 4), v_job(st)))
            for sc in range(1, NSC):
                jobs.append((4 * sc, sc, qk_job(KT[0], w_sb["wk"], 0, sc)))
            for g in range(1, NG):
                for sc in range(NSC):
                    jobs.append((16 * g + 4 * sc, sc + 1, qk_job(KT[g], w_sb["wk"], g, sc)))
                for sc in range(2):
                    jobs.append((16 * g, sc + 1, qk_job(QT[g], w_sb["wq"], g, sc)))
            for g in range(NG):
                for sc in range(2, NSC):
                    jobs.append((16 * (4 + g), sc + 1, qk_job(QT[g], w_sb["wq"], g, sc)))
            # output projection for the first query half: ready once all qh0
            # blocks are normed (~2 ticks after qh0 ends); no hard deadline.
            WO_INJECT = True
            if WO_INJECT:
                for st in range(8):
                    for ob in range(2):
                        jobs.append((1 << 29, 16 * 4 + 3, wo_job(st, ob)))
            jobs.sort(key=lambda j: j[0])
            for d, r, gen in jobs:
                filler.add(d, r, gen)

            # ---------------- attention driver: pending FIFO carries the
            # V/colsum matmuls LAG ticks behind their exps (deep in block 0 to
            # spread the V-projection burst, depth 1 afterwards).
            pending = []
            tick = 0
            for qh in range(2):
                for g in range(NG):
                    blk = AttnBlock(g, qh)
                    for kt in range(NKT):
                        blk.emit_scores_exp(kt)
                        lag = 7 if tick < 8 else max(1, 14 - tick)
                        while len(pending) > lag:
                            b, k = pending.pop(0)
                            b.emit_v_cs(k)
                        pending.append((blk, kt))
                        filler.tick(tick, budget=3)
                        tick += 1
            while pending:
                b, k = pending.pop(0)
                b.emit_v_cs(k)
            filler.drain()

            # ---------------- output projection, second query half
            for st in (range(NKT) if not WO_INJECT else range(8, NKT)):
                for ob in range(2):
                    for _ in wo_job(st, ob):
                        pass

    _split_sync_waits(nc)
    return nc


_NC = None


def _get_nc():
    global _NC
    if _NC is None:
        _NC = build_nc()
    return _NC


# ---------------------------------------------------------------- host side
def make_in_maps(x, wq, wk, wv, wo):
    x = np.asarray(x, dtype=np.float32)
    wq = np.asarray(wq, dtype=np.float32)
    wk = np.asarray(wk, dtype=np.float32)
    wv = np.asarray(wv, dtype=np.float32)
    wo = np.asarray(wo, dtype=np.float32)
    in_maps = []
    for c in range(N_CORES):
        b, hg = c // 2, c % 2
        sl = slice(hg * DL, (hg + 1) * DL)
        xTc = np.ascontiguousarray(x[b].T).astype(BF16_NP)
        wqTc = np.ascontiguousarray((wq[sl] / 8.0).T).astype(BF16_NP)
        wkTc = np.ascontiguousarray(wk[sl].T).astype(BF16_NP)
        wvTc = np.ascontiguousarray(wv[sl].T).astype(BF16_NP)
        woTc = np.ascontiguousarray(wo[:, sl].T).astype(BF16_NP)
        in_maps.append(
            {"xT": xTc, "wqT": wqTc, "wkT": wkTc, "wvT": wvTc, "woT": woTc}
        )
    return in_maps


def gather(results):
    out = np.zeros((4, S, DM), dtype=np.float32)
    for c in range(N_CORES):
        out[c // 2] += results[c]["out"]
    return out


def kernel(x, wq, wk, wv, wo):
    from concourse.bass_utils import run_bass_kernel_spmd

    nc = _get_nc()
    in_maps = make_in_maps(x, wq, wk, wv, wo)
    res = run_bass_kernel_spmd(nc, in_maps, CORE_IDS)
    return gather(res.results)


# revision 19
# speedup vs baseline: 1.1964x; 1.1964x over previous
"""Multi-head self-attention (B=4, S=2048, D=1024, H=16) on 8 trn2 NeuronCores.

Sharding: batch (4) x head-group (2 groups of 8 heads) -> 8 cores.
Each core computes, for its (batch b, head-group hg):
  Q'^T = (wq_l/8) @ x_b^T            [512, 2048]   (1/sqrt(dk) folded into wq)
  K^T  = wk_l @ x_b^T                [512, 2048]
  V    = x_b @ wv_l^T                [2048, 512]
  per head h (8 local, dk=64), in transposed layout (keys on partitions):
    scoresT[k, q] = K_h @ Q'_h^T     (no max-subtraction: scores ~ N(0,4), exp
                                      of |s|<~12 is safe in fp32/bf16)
    expT = exp(scoresT)              (ScalarE, PSUM->SBUF bf16)
    unnormT[c, q] = V_h^T @ expT     (PE, accumulated over key tiles)
    Z[q] = ones^T @ expT             (PE colsum quads, same accumulation)
    attnT = unnormT / Z              (reciprocal once + DMA partition
                                      broadcast via DRAM + DVE muls)
  out_partial = attnT^T @ wo_l^T     [2048, 1024]  (row-parallel wo)
Host sums the two partials per batch (the "all-reduce" of row-parallel wo).

v2 schedule: the 256 exps (ScalarE, ~1.11us each) are the pacer.  Blocks run
qh-outer (all 4 head-pairs for queries 0-1023, then 1024-2047) so the output
projection for the first query half can overlap the second attention phase.
All projection work except the first three Q/K chunks is deferred into the
attention stream as fine-grained (2-matmul) filler pieces pulled by a
deadline-driven queue, so the PE never blocks the exp chain for more than
~0.5us at a time.  DMA is staged (wk+xq0 first, then wq+wv, then the rest) so
the first scores matmul can issue at ~15us.
"""

import ml_dtypes
import numpy as np

import bass_rust
import concourse.bass as bass
import concourse.mybir as mybir
import concourse.tile as tile

# ---------------------------------------------------------------- constants
S = 2048          # sequence length
DM = 1024         # model dim
DL = 512          # local (per-core) head dims = 8 heads * 64
DK = 64           # head dim
P = 128
NKT = S // P      # 16 key tiles
NG = DL // P      # 4 head-pairs (c-tiles / dq-tiles)
KD = DM // P      # 8 contraction tiles for projections
NSC = S // 512    # 4 s-chunks for projections
F32 = mybir.dt.float32
BF16 = mybir.dt.bfloat16
BF16_NP = ml_dtypes.bfloat16

N_CORES = 8
CORE_IDS = list(range(N_CORES))


# ------------------------------------------------- walrus sync-wait workaround
def _split_sync_waits(nc, limit=1):
    """This toolchain's walrus codegen rejects instructions carrying more than
    one sync-wait command.  Move excess waits onto dedicated same-engine nops
    inserted immediately before the instruction (sequential waits on the same
    engine queue are semantically identical to multiple waits on one inst)."""
    fn = nc.m.functions[0]
    snapshots = [(bb, list(bb.instructions)) for bb in fn.blocks]
    plans = []
    for _bb, insts in snapshots:
        plan = {}
        for idx, inst in enumerate(insts):
            si = inst.sync_info
            waits = list(si.on_wait) if si and si.on_wait else []
            if len(waits) > limit:
                pre, keep = waits[:-limit], waits[-limit:]
                nops = []
                for w in pre:
                    ni = nc.engines[inst.engine].nop(nofuse=True, hint="wsplit").ins
                    ni.sync_info = bass_rust.SyncInfo(on_wait=[w], on_update=[])
                    nops.append(ni)
                si.on_wait = keep
                plan[idx] = nops
        plans.append(plan)
    # Rebuild every block from its pre-pass snapshot plus insertions; this also
    # drops the fresh nops from wherever bass appended them at creation time.
    for (bb, insts), plan in zip(snapshots, plans):
        out = []
        for idx, inst in enumerate(insts):
            out.extend(plan.get(idx, ()))
            out.append(inst)
        bb.instructions = out


# ---------------------------------------------------------------- the program
def build_nc():
    """Build the SPMD per-core Bass program (identical on all 8 cores)."""
    nc = bass.Bass()

    xT = nc.declare_dram_parameter("xT", [DM, S], BF16, isOutput=False)
    wqT = nc.declare_dram_parameter("wqT", [DM, DL], BF16, isOutput=False)
    wkT = nc.declare_dram_parameter("wkT", [DM, DL], BF16, isOutput=False)
    wvT = nc.declare_dram_parameter("wvT", [DM, DL], BF16, isOutput=False)
    woT = nc.declare_dram_parameter("woT", [DL, DM], BF16, isOutput=False)
    out = nc.declare_dram_parameter("out", [S, DM], F32, isOutput=True)

    with tile.TileContext(nc) as tc:
        with (
            tc.tile_pool(name="big", bufs=1) as big,
            tc.tile_pool(name="expT", bufs=16) as expp,
            tc.tile_pool(name="rc", bufs=2) as rcp,
            tc.tile_pool(name="outsb", bufs=3) as outp,
            tc.tile_pool(name="dram", bufs=2, space="DRAM") as dramp,
            tc.tile_pool(name="ps", bufs=2, space="PSUM") as psp,
            tc.tile_pool(name="acc", bufs=1, space="PSUM") as accp,
            tc.tile_pool(name="fil", bufs=1, space="PSUM") as filp,
        ):
            # ---------------- staged DRAM loads.  Stage 1 (wk + first x
            # quarter) ships alone so the first K/Q projection chunks can
            # start at ~15us; later stages are gated behind earlier tensors
            # via 1-element gpsimd copies (RAW on the gating tile, WAW on the
            # staged destination) so they don't steal HBM bandwidth early.
            w_sb = {}
            for name in ("wk", "wq", "wv"):
                w_sb[name] = big.tile([P, KD, DL], BF16, tag=name, name=name)
            xT_r = xT.rearrange("(kd p) s -> p kd s", p=P)
            xT_q = [
                big.tile([P, KD, 512], BF16, tag=f"xT{j}", name=f"xTq{j}")
                for j in range(4)
            ]
            woT_sb = big.tile([P, NG, DM], BF16, tag="wo")

            def stage(dsts, gate):
                if gate is not None:
                    for d in dsts:
                        nc.gpsimd.tensor_copy(
                            out=d[0:1, 0, 0:1], in_=gate[0:1, 0, 0:1]
                        )
            # stage 1: wk, wq, xq0 (feeds the upfront K0/Q0 chunks)
            stage([w_sb["wk"], w_sb["wq"], xT_q[0]], None)
            nc.sync.dma_start(
                w_sb["wk"][:], wkT.rearrange("(kd p) m -> p kd m", p=P)
            )
            nc.sync.dma_start(
                w_sb["wq"][:], wqT.rearrange("(kd p) m -> p kd m", p=P)
            )
            nc.sync.dma_start(xT_q[0][:], xT_r[:, :, 0:512])
            # stage 2: wv + xq1 (gated on wk): Q0 sc1 reads x cols 512-1023
            stage([w_sb["wv"], xT_q[1]], w_sb["wk"])
            nc.sync.dma_start(
                w_sb["wv"][:], wvT.rearrange("(kd p) m -> p kd m", p=P)
            )
            nc.sync.dma_start(xT_q[1][:], xT_r[:, :, 512:1024])
            # stage 3: xq2, then xq3+wo chained behind
            stage([xT_q[2]], xT_q[1])
            nc.sync.dma_start(xT_q[2][:], xT_r[:, :, 1024:1536])
            stage([xT_q[3], woT_sb], xT_q[2])
            nc.sync.dma_start(xT_q[3][:], xT_r[:, :, 1536:2048])
            nc.sync.dma_start(woT_sb[:], woT.rearrange("(ct p) o -> p ct o", p=P))

            def xslice(kd, fr, to):
                q = fr // 512
                assert to <= (q + 1) * 512
                return xT_q[q][:, kd, fr - q * 512 : to - q * 512]

            # ---------------- constants
            ones_bf = big.tile([P, 1], BF16, tag="ones")
            nc.vector.memset(ones_bf[:], 1.0)
            warm = big.tile([P, 512], BF16, tag="warm")
            nc.vector.memset(warm[:], 0.0)

            # HAM warm-up: keep the PE busy on throwaway matmuls while the
            # stage-1 DMA lands so the first real chunks run at 2.4GHz.
            ps_w = psp.tile([P, 512], F32, tag="ps", name="warmps")
            for _ in range(24):
                nc.tensor.matmul(
                    ps_w[:], lhsT=warm[:, 0:128], rhs=warm[:], start=True,
                    stop=True,
                )

            # persistent activation tensors
            QT = [big.tile([P, S], BF16, tag=f"QT{g}", name=f"QT{g}") for g in range(NG)]
            KT = [big.tile([P, S], BF16, tag=f"KT{g}", name=f"KT{g}") for g in range(NG)]
            V_st = [big.tile([P, 8, DK + 1], BF16, tag=f"V{st}", name=f"V{st}") for st in range(NKT)]
            attn = [big.tile([P, S], BF16, tag=f"attn{g}", name=f"attn{g}") for g in range(NG)]

            # ---------------- filler machinery: all projection / output work
            # is expressed as jobs that yield 2-matmul pieces; the attention
            # driver pulls pieces by deadline so the exp chain never waits
            # long on the PE queue.
            def qk_job(dst, w, g, sc):
                ps = filp.tile([P, 512], F32, tag="fil", name="projch")
                for kd0 in range(0, KD, 2):
                    for kd in (kd0, kd0 + 1):
                        nc.tensor.matmul(
                            ps[:],
                            lhsT=w[:, kd, g * P : (g + 1) * P],
                            rhs=xslice(kd, sc * 512, (sc + 1) * 512),
                            start=(kd == 0),
                            stop=(kd == KD - 1),
                        )
                    yield
                nc.vector.tensor_copy(
                    out=dst[:, sc * 512 : (sc + 1) * 512], in_=ps[:]
                )

            def v_job(st):
                ps = filp.tile([P, 512], F32, tag="fil", name="vch")
                for kd0 in range(0, KD, 2):
                    for kd in (kd0, kd0 + 1):
                        nc.tensor.matmul(
                            ps[:],
                            lhsT=xslice(kd, st * P, (st + 1) * P),
                            rhs=w_sb["wv"][:, kd, :],
                            start=(kd == 0),
                            stop=(kd == KD - 1),
                        )
                    yield
                nc.vector.tensor_copy(
                    out=V_st[st][:, :, 0:DK],
                    in_=ps.rearrange("p (h c) -> p h c", c=DK),
                )

            def wo_job(st, ob):
                ps = filp.tile([P, 512], F32, tag="fil", name="wochunk")
                for ct0 in (0, 2):
                    for ct in (ct0, ct0 + 1):
                        nc.tensor.matmul(
                            ps[:],
                            lhsT=attn[ct][:, st * P : (st + 1) * P],
                            rhs=woT_sb[:, ct, ob * 512 : (ob + 1) * 512],
                            start=(ct == 0),
                            stop=(ct == NG - 1),
                        )
                    yield
                ot = outp.tile([P, 512], F32, tag="out")
                nc.vector.tensor_copy(out=ot[:], in_=ps[:])
                nc.sync.dma_start(
                    out[st * P : (st + 1) * P, ob * 512 : (ob + 1) * 512], ot[:]
                )

            class Filler:
                """Ordered queue of (deadline_tick, ready_tick, job-generator).
                One job open at a time (so filler holds a single av psum
                slot); pieces are pulled per tick: everything past deadline
                unconditionally, plus up to `budget` opportunistic pieces."""

                def __init__(self):
                    self.jobs = []
                    self.open = None
                    self.open_deadline = 0

                def add(self, deadline, ready, gen):
                    self.jobs.append((deadline, ready, gen))

                def _pull_one(self, tick):
                    if self.open is None:
                        if not self.jobs or self.jobs[0][1] > tick:
                            return False
                        self.open_deadline, _, self.open = self.jobs.pop(0)
                    try:
                        next(self.open)
                    except StopIteration:
                        self.open = None
                    return True

                def tick(self, tick, budget=2):
                    n = 0
                    while True:
                        urgent = (
                            self.open is not None and self.open_deadline <= tick + 1
                        ) or (
                            self.open is None
                            and self.jobs
                            and self.jobs[0][0] <= tick + 1
                            and self.jobs[0][1] <= tick
                        )
                        if not urgent and n >= budget:
                            break
                        if not self._pull_one(tick):
                            break
                        n += 1

                def drain(self):
                    while self._pull_one(1 << 30):
                        pass

            filler = Filler()

            # ---------------- attention
            class AttnBlock:
                """Heads A=2g, B=2g+1; query half qh (1024 queries).

                scoresT/exp are ACT-paced.  V matmuls run a few kt behind
                (pending FIFO in the driver) so both heads' exp tiles are
                ready together, letting adjacently issued matmuls with
                disjoint array col groups (V: 0-1 vs 2-3) run concurrently
                on the PE.  vt accumulates A in partitions 0-63 and B in
                64-127 of one bank (memset + start=False keeps the
                interleaved accumulation groups from clearing each other).
                The softmax denominators are accumulated OFF the PE: per-kt
                elementwise adds of the exp tiles into a [128,1024] bf16
                accumulator (GpSimd for head A, DVE for head B), reduced
                across partitions by a single 4-matmul quad at block end.
                Normalization runs entirely off the critical path.
                """

                def __init__(self, g, qh):
                    self.g, self.qoff = g, qh * 1024
                    self.vt = [
                        accp.tile([P, 512], F32, tag=f"vt{qb}", name=f"vt{qb}")
                        for qb in range(2)
                    ]
                    self.cs = accp.tile([P, 512], F32, tag="cs")
                    for t in self.vt:
                        nc.vector.memset(t[:], 0.0)
                    nc.vector.memset(self.cs[:], 0.0)
                    self.ets = {}

                def emit_scores_exp(self, kt):
                    g, qoff = self.g, self.qoff
                    for hp, pb in ((0, 0), (1, 64)):
                        ps_s = psp.tile([P, 1024], F32, tag="ps", name=f"ps_s{hp}")
                        for qb in range(2):
                            nc.tensor.matmul(
                                ps_s[:, qb * 512 : (qb + 1) * 512],
                                lhsT=KT[g][pb : pb + 64, kt * P : (kt + 1) * P],
                                rhs=QT[g][
                                    pb : pb + 64,
                                    qoff + qb * 512 : qoff + (qb + 1) * 512,
                                ],
                                start=True,
                                stop=True,
                            )
                        et = expp.tile([P, 1024], BF16, tag="expT", name=f"et{hp}")
                        nc.scalar.activation(
                            et[:], ps_s[:], mybir.ActivationFunctionType.Exp
                        )
                        self.ets[(kt, hp)] = et

                def emit_v(self, kt):
                    g = self.g
                    last = kt == NKT - 1
                    et = self.ets[kt] = {
                        hp: self.ets.pop((kt, hp)) for hp in (0, 1)
                    }
                    for qb in range(2):
                        for hp, pb in ((0, 0), (1, 64)):
                            nc.tensor.matmul(
                                self.vt[qb][pb : pb + 64, :],
                                lhsT=V_st[kt][:, 2 * g + hp, 0:DK],
                                rhs=et[hp][:, qb * 512 : (qb + 1) * 512],
                                start=False,
                                stop=last,
                                skip_group_check=True,
                                tile_position=(0, pb),
                            )

                def emit_cs(self, kt):
                    last = kt == NKT - 1
                    et = self.ets.pop(kt)
                    for hp in (0, 1):
                        for qb in range(2):
                            cp = 64 * hp + 32 * qb
                            nc.tensor.matmul(
                                self.cs[cp : cp + 1, :],
                                lhsT=ones_bf[:],
                                rhs=et[hp][:, qb * 512 : (qb + 1) * 512],
                                start=False,
                                stop=last,
                                skip_group_check=True,
                                tile_position=(0, cp),
                            )
                    if last:
                        self.emit_norm()

                def emit_norm(self):
                    g, qoff = self.g, self.qoff
                    un = [
                        rcp.tile([P, 512], F32, tag=f"un{qb}", name=f"un{qb}")
                        for qb in range(2)
                    ]
                    for qb in range(2):
                        nc.vector.tensor_copy(out=un[qb][:], in_=self.vt[qb][:])
                    cs_sb = rcp.tile([P, 512], F32, tag="cs_sb")
                    nc.vector.tensor_copy(out=cs_sb[:], in_=self.cs[:])
                    zd = dramp.tile([4, 512], F32, name="zd")
                    # zd rows: 0=(A,qb0) 1=(A,qb1) 2=(B,qb0) 3=(B,qb1)
                    nc.sync.dma_start(zd[:], cs_sb[0:128:32, :])
                    # reciprocal on a [128,16] reshape of the 2048 real Z
                    # values (vs [128,512]: DVE reciprocal is ~8 cyc/col)
                    zs = rcp.tile([P, 16], F32, tag="zs")
                    nc.sync.dma_start(
                        zs[:], zd.rearrange("a (b c) -> (a b) c", c=16)
                    )
                    zr = rcp.tile([P, 16], F32, tag="zr")
                    nc.vector.reciprocal(zr[:], zs[:])
                    zd2 = dramp.tile([4, 512], F32, name="zd2")
                    nc.sync.dma_start(
                        zd2.rearrange("a (b c) -> (a b) c", c=16), zr[:]
                    )
                    for qb in range(2):
                        rcb = rcp.tile(
                            [P, 512], F32, tag=f"rcb{qb}", name=f"rcb{qb}"
                        )
                        nc.sync.dma_start(
                            rcb[0:64, :], zd2[qb, None, :].to_broadcast([64, 512])
                        )
                        nc.sync.dma_start(
                            rcb[64:128, :],
                            zd2[qb + 2, None, :].to_broadcast([64, 512]),
                        )
                        for pb in (0, 64):
                            nc.vector.tensor_mul(
                                out=attn[g][
                                    pb : pb + 64,
                                    qoff + qb * 512 : qoff + (qb + 1) * 512,
                                ],
                                in0=un[qb][pb : pb + 64, :],
                                in1=rcb[pb : pb + 64, :],
                            )

            # ---------------- upfront projections: just enough for the first
            # scores matmul (K0 cols 0-511, Q0 cols 0-1023)
            for job in (
                qk_job(KT[0], w_sb["wk"], 0, 0),
                qk_job(QT[0], w_sb["wq"], 0, 0),
                qk_job(QT[0], w_sb["wq"], 0, 1),
            ):
                for _ in job:
                    pass

            # ---------------- filler job schedule (deadlines in global ticks;
            # 1 tick = one (block, kt) step = ~2.2us of exp pacing).
            # Block order: qh-outer -> block index B = qh*4 + g.
            # V_st[st] feeds emit_v_cs at tick st + LAG of block 0.
            # K[g][sc] feeds scores of block g (qh0) at kt=4*sc.
            # Q[g][qh sc] feeds block qh*4+g from its start.
            jobs = []
            # V_st[st] is consumed by the block-0 v_cs pop at pop_tick[st];
            # derive that from the same lag schedule the driver uses so the
            # projection is always fully emitted before its consumer.
            def lag_at(t):
                if t < 8:
                    return 7
                if t < 13:
                    return 14 - t
                return 1 if t % 2 else 2

            pop_tick = {}
            sim_pending, t = [], 0
            while len(pop_tick) < NKT:
                while len(sim_pending) > lag_at(t):
                    pop_tick[sim_pending.pop(0)] = t
                sim_pending.append(t)
                t += 1
            for st in range(NKT):
                jobs.append((pop_tick[st] - 1, max(0, st // 4), v_job(st)))
            for sc in range(1, NSC):
                jobs.append((4 * sc, sc, qk_job(KT[0], w_sb["wk"], 0, sc)))
            for g in range(1, NG):
                for sc in range(NSC):
                    jobs.append((16 * g + 4 * sc, sc + 1, qk_job(KT[g], w_sb["wk"], g, sc)))
                for sc in range(2):
                    jobs.append((16 * g, sc + 1, qk_job(QT[g], w_sb["wq"], g, sc)))
            for g in range(NG):
                for sc in range(2, NSC):
                    jobs.append((16 * (4 + g), sc + 1, qk_job(QT[g], w_sb["wq"], g, sc)))
            # output projection for the first query half: ready once all qh0
            # blocks are normed (~2 ticks after qh0 ends); no hard deadline.
            WO_INJECT = True
            if WO_INJECT:
                for st in range(8):
                    for ob in range(2):
                        jobs.append((1 << 29, 16 * 4 + 3, wo_job(st, ob)))
            jobs.sort(key=lambda j: j[0])
            for d, r, gen in jobs:
                filler.add(d, r, gen)

            # ---------------- attention driver: pending FIFO carries the
            # V/colsum matmuls LAG ticks behind their exps (deep in block 0 to
            # spread the V-projection burst, paired afterwards so the V and
            # colsum matmul groups batch two kt at a time -- every PE weight-
            # geometry switch costs a ~160ns pipeline-drain bubble, so fewer,
            # larger same-shape groups waste less).  Fillers likewise burst on
            # odd ticks only (urgent deadline work still goes out every tick).
            def emit_batch(popped):
                for b, k in popped:
                    b.emit_v(k)
                for b, k in popped:
                    b.emit_cs(k)

            pending = []
            tick = 0
            for qh in range(2):
                for g in range(NG):
                    blk = AttnBlock(g, qh)
                    for kt in range(NKT):
                        blk.emit_scores_exp(kt)
                        lag = lag_at(tick)
                        popped = []
                        while len(pending) > lag:
                            popped.append(pending.pop(0))
                        emit_batch(popped)
                        pending.append((blk, kt))
                        filler.tick(tick, budget=6 if tick % 2 else 0)
                        tick += 1
            while pending:
                emit_batch(pending[:2])
                del pending[:2]
            filler.drain()

            # ---------------- output projection, second query half
            for st in (range(NKT) if not WO_INJECT else range(8, NKT)):
                for ob in range(2):
                    for _ in wo_job(st, ob):
                        pass

    _split_sync_waits(nc)
    return nc


_NC = None


def _get_nc():
    global _NC
    if _NC is None:
        _NC = build_nc()
    return _NC


# ---------------------------------------------------------------- host side
def make_in_maps(x, wq, wk, wv, wo):
    x = np.asarray(x, dtype=np.float32)
    wq = np.asarray(wq, dtype=np.float32)
    wk = np.asarray(wk, dtype=np.float32)
    wv = np.asarray(wv, dtype=np.float32)
    wo = np.asarray(wo, dtype=np.float32)
    in_maps = []
    for c in range(N_CORES):
        b, hg = c // 2, c % 2
        sl = slice(hg * DL, (hg + 1) * DL)
        xTc = np.ascontiguousarray(x[b].T).astype(BF16_NP)
        wqTc = np.ascontiguousarray((wq[sl] / 8.0).T).astype(BF16_NP)
        wkTc = np.ascontiguousarray(wk[sl].T).astype(BF16_NP)
        wvTc = np.ascontiguousarray(wv[sl].T).astype(BF16_NP)
        woTc = np.ascontiguousarray(wo[:, sl].T).astype(BF16_NP)
        in_maps.append(
            {"xT": xTc, "wqT": wqTc, "wkT": wkTc, "wvT": wvTc, "woT": woTc}
        )
    return in_maps


def gather(results):
    out = np.zeros((4, S, DM), dtype=np.float32)
    for c in range(N_CORES):
        out[c // 2] += results[c]["out"]
    return out


def kernel(x, wq, wk, wv, wo):
    from concourse.bass_utils import run_bass_kernel_spmd

    nc = _get_nc()
    in_maps = make_in_maps(x, wq, wk, wv, wo)
    res = run_bass_kernel_spmd(nc, in_maps, CORE_IDS)
    return gather(res.results)


# revision 21
# speedup vs baseline: 1.1996x; 1.0026x over previous
"""Multi-head self-attention (B=4, S=2048, D=1024, H=16) on 8 trn2 NeuronCores.

Sharding: batch (4) x head-group (2 groups of 8 heads) -> 8 cores.
Each core computes, for its (batch b, head-group hg):
  Q'^T = (wq_l/8) @ x_b^T            [512, 2048]   (1/sqrt(dk) folded into wq)
  K^T  = wk_l @ x_b^T                [512, 2048]
  V    = x_b @ wv_l^T                [2048, 512]
  per head h (8 local, dk=64), in transposed layout (keys on partitions):
    scoresT[k, q] = K_h @ Q'_h^T     (no max-subtraction: scores ~ N(0,4), exp
                                      of |s|<~12 is safe in fp32/bf16)
    expT = exp(scoresT)              (ScalarE, PSUM->SBUF bf16)
    unnormT[c, q] = V_h^T @ expT     (PE, accumulated over key tiles)
    Z[q] = ones^T @ expT             (PE colsum quads, same accumulation)
    attnT = unnormT / Z              (reciprocal once + DMA partition
                                      broadcast via DRAM + DVE muls)
  out_partial = attnT^T @ wo_l^T     [2048, 1024]  (row-parallel wo)
Host sums the two partials per batch (the "all-reduce" of row-parallel wo).

v2 schedule: the 256 exps (ScalarE, ~1.11us each) are the pacer.  Blocks run
qh-outer (all 4 head-pairs for queries 0-1023, then 1024-2047) so the output
projection for the first query half can overlap the second attention phase.
All projection work except the first three Q/K chunks is deferred into the
attention stream as fine-grained (2-matmul) filler pieces pulled by a
deadline-driven queue, so the PE never blocks the exp chain for more than
~0.5us at a time.  DMA is staged (wk+xq0 first, then wq+wv, then the rest) so
the first scores matmul can issue at ~15us.
"""

import ml_dtypes
import numpy as np

import bass_rust
import concourse.bass as bass
import concourse.mybir as mybir
import concourse.tile as tile

# ---------------------------------------------------------------- constants
S = 2048          # sequence length
DM = 1024         # model dim
DL = 512          # local (per-core) head dims = 8 heads * 64
DK = 64           # head dim
P = 128
NKT = S // P      # 16 key tiles
NG = DL // P      # 4 head-pairs (c-tiles / dq-tiles)
KD = DM // P      # 8 contraction tiles for projections
NSC = S // 512    # 4 s-chunks for projections
F32 = mybir.dt.float32
BF16 = mybir.dt.bfloat16
BF16_NP = ml_dtypes.bfloat16

N_CORES = 8
CORE_IDS = list(range(N_CORES))


# ------------------------------------------------- walrus sync-wait workaround
def _split_sync_waits(nc, limit=1):
    """This toolchain's walrus codegen rejects instructions carrying more than
    one sync-wait command.  Move excess waits onto dedicated same-engine nops
    inserted immediately before the instruction (sequential waits on the same
    engine queue are semantically identical to multiple waits on one inst)."""
    fn = nc.m.functions[0]
    snapshots = [(bb, list(bb.instructions)) for bb in fn.blocks]
    plans = []
    for _bb, insts in snapshots:
        plan = {}
        for idx, inst in enumerate(insts):
            si = inst.sync_info
            waits = list(si.on_wait) if si and si.on_wait else []
            if len(waits) > limit:
                pre, keep = waits[:-limit], waits[-limit:]
                nops = []
                for w in pre:
                    ni = nc.engines[inst.engine].nop(nofuse=True, hint="wsplit").ins
                    ni.sync_info = bass_rust.SyncInfo(on_wait=[w], on_update=[])
                    nops.append(ni)
                si.on_wait = keep
                plan[idx] = nops
        plans.append(plan)
    # Rebuild every block from its pre-pass snapshot plus insertions; this also
    # drops the fresh nops from wherever bass appended them at creation time.
    for (bb, insts), plan in zip(snapshots, plans):
        out = []
        for idx, inst in enumerate(insts):
            out.extend(plan.get(idx, ()))
            out.append(inst)
        bb.instructions = out


# ---------------------------------------------------------------- the program
def build_nc():
    """Build the SPMD per-core Bass program (identical on all 8 cores)."""
    nc = bass.Bass()

    xT = nc.declare_dram_parameter("xT", [DM, S], BF16, isOutput=False)
    wqT = nc.declare_dram_parameter("wqT", [DM, DL], BF16, isOutput=False)
    wkT = nc.declare_dram_parameter("wkT", [DM, DL], BF16, isOutput=False)
    wvT = nc.declare_dram_parameter("wvT", [DM, DL], BF16, isOutput=False)
    woT = nc.declare_dram_parameter("woT", [DL, DM], BF16, isOutput=False)
    out = nc.declare_dram_parameter("out", [S, DM], F32, isOutput=True)

    with tile.TileContext(nc) as tc:
        with (
            tc.tile_pool(name="big", bufs=1) as big,
            tc.tile_pool(name="expT", bufs=16) as expp,
            tc.tile_pool(name="rc", bufs=2) as rcp,
            tc.tile_pool(name="outsb", bufs=3) as outp,
            tc.tile_pool(name="dram", bufs=2, space="DRAM") as dramp,
            tc.tile_pool(name="ps", bufs=2, space="PSUM") as psp,
            tc.tile_pool(name="acc", bufs=1, space="PSUM") as accp,
            tc.tile_pool(name="fil", bufs=1, space="PSUM") as filp,
        ):
            # ---------------- staged DRAM loads.  Stage 1 (wk + first x
            # quarter) ships alone so the first K/Q projection chunks can
            # start at ~15us; later stages are gated behind earlier tensors
            # via 1-element gpsimd copies (RAW on the gating tile, WAW on the
            # staged destination) so they don't steal HBM bandwidth early.
            w_sb = {}
            for name in ("wk", "wq", "wv"):
                w_sb[name] = big.tile([P, KD, DL], BF16, tag=name, name=name)
            xT_r = xT.rearrange("(kd p) s -> p kd s", p=P)
            xT_q = [
                big.tile([P, KD, 512], BF16, tag=f"xT{j}", name=f"xTq{j}")
                for j in range(4)
            ]
            woT_sb = big.tile([P, NG, DM], BF16, tag="wo")

            def stage(dsts, gate):
                if gate is not None:
                    for d in dsts:
                        nc.gpsimd.tensor_copy(
                            out=d[0:1, 0, 0:1], in_=gate[0:1, 0, 0:1]
                        )
            # stage 1: wk, wq, xq0 (feeds the upfront K0/Q0 chunks)
            stage([w_sb["wk"], w_sb["wq"], xT_q[0]], None)
            nc.sync.dma_start(
                w_sb["wk"][:], wkT.rearrange("(kd p) m -> p kd m", p=P)
            )
            nc.sync.dma_start(
                w_sb["wq"][:], wqT.rearrange("(kd p) m -> p kd m", p=P)
            )
            nc.sync.dma_start(xT_q[0][:], xT_r[:, :, 0:512])
            # stage 2: wv + xq1 (gated on wk): Q0 sc1 reads x cols 512-1023
            stage([w_sb["wv"], xT_q[1]], w_sb["wk"])
            nc.sync.dma_start(
                w_sb["wv"][:], wvT.rearrange("(kd p) m -> p kd m", p=P)
            )
            nc.sync.dma_start(xT_q[1][:], xT_r[:, :, 512:1024])
            # stage 3: xq2, then xq3+wo chained behind
            stage([xT_q[2]], xT_q[1])
            nc.sync.dma_start(xT_q[2][:], xT_r[:, :, 1024:1536])
            stage([xT_q[3], woT_sb], xT_q[2])
            nc.sync.dma_start(xT_q[3][:], xT_r[:, :, 1536:2048])
            nc.sync.dma_start(woT_sb[:], woT.rearrange("(ct p) o -> p ct o", p=P))

            def xslice(kd, fr, to):
                q = fr // 512
                assert to <= (q + 1) * 512
                return xT_q[q][:, kd, fr - q * 512 : to - q * 512]

            # ---------------- constants
            ones_bf = big.tile([P, 1], BF16, tag="ones")
            nc.vector.memset(ones_bf[:], 1.0)
            warm = big.tile([P, 512], BF16, tag="warm")
            nc.vector.memset(warm[:], 0.0)

            # HAM warm-up: keep the PE busy on throwaway matmuls while the
            # stage-1 DMA lands so the first real chunks run at 2.4GHz.
            ps_w = psp.tile([P, 512], F32, tag="ps", name="warmps")
            for _ in range(24):
                nc.tensor.matmul(
                    ps_w[:], lhsT=warm[:, 0:128], rhs=warm[:], start=True,
                    stop=True,
                )

            # persistent activation tensors
            QT = [big.tile([P, S], BF16, tag=f"QT{g}", name=f"QT{g}") for g in range(NG)]
            KT = [big.tile([P, S], BF16, tag=f"KT{g}", name=f"KT{g}") for g in range(NG)]
            V_st = [big.tile([P, 8, DK + 1], BF16, tag=f"V{st}", name=f"V{st}") for st in range(NKT)]
            attn = [big.tile([P, S], BF16, tag=f"attn{g}", name=f"attn{g}") for g in range(NG)]

            # ---------------- filler machinery: all projection / output work
            # is expressed as jobs that yield 2-matmul pieces; the attention
            # driver pulls pieces by deadline so the exp chain never waits
            # long on the PE queue.
            def qk_job(dst, w, g, sc):
                ps = filp.tile([P, 512], F32, tag="fil", name="projch")
                for kd0 in range(0, KD, 4):
                    for kd in range(kd0, kd0 + 4):
                        nc.tensor.matmul(
                            ps[:],
                            lhsT=w[:, kd, g * P : (g + 1) * P],
                            rhs=xslice(kd, sc * 512, (sc + 1) * 512),
                            start=(kd == 0),
                            stop=(kd == KD - 1),
                        )
                    yield
                nc.vector.tensor_copy(
                    out=dst[:, sc * 512 : (sc + 1) * 512], in_=ps[:]
                )

            def v_job(st):
                ps = filp.tile([P, 512], F32, tag="fil", name="vch")
                for kd0 in range(0, KD, 4):
                    for kd in range(kd0, kd0 + 4):
                        nc.tensor.matmul(
                            ps[:],
                            lhsT=xslice(kd, st * P, (st + 1) * P),
                            rhs=w_sb["wv"][:, kd, :],
                            start=(kd == 0),
                            stop=(kd == KD - 1),
                        )
                    yield
                nc.vector.tensor_copy(
                    out=V_st[st][:, :, 0:DK],
                    in_=ps.rearrange("p (h c) -> p h c", c=DK),
                )

            def wo_job(st, ob):
                ps = filp.tile([P, 512], F32, tag="fil", name="wochunk")
                for ct0 in (0, 2):
                    for ct in (ct0, ct0 + 1):
                        nc.tensor.matmul(
                            ps[:],
                            lhsT=attn[ct][:, st * P : (st + 1) * P],
                            rhs=woT_sb[:, ct, ob * 512 : (ob + 1) * 512],
                            start=(ct == 0),
                            stop=(ct == NG - 1),
                        )
                    yield
                ot = outp.tile([P, 512], F32, tag="out")
                nc.vector.tensor_copy(out=ot[:], in_=ps[:])
                nc.sync.dma_start(
                    out[st * P : (st + 1) * P, ob * 512 : (ob + 1) * 512], ot[:]
                )

            class Filler:
                """Ordered queue of (deadline_tick, ready_tick, job-generator).
                One job open at a time (so filler holds a single av psum
                slot); pieces are pulled per tick: everything past deadline
                unconditionally, plus up to `budget` opportunistic pieces."""

                def __init__(self):
                    self.jobs = []
                    self.open = None
                    self.open_deadline = 0

                def add(self, deadline, ready, gen):
                    self.jobs.append((deadline, ready, gen))

                def _pull_one(self, tick):
                    if self.open is None:
                        if not self.jobs or self.jobs[0][1] > tick:
                            return False
                        self.open_deadline, _, self.open = self.jobs.pop(0)
                    try:
                        next(self.open)
                    except StopIteration:
                        self.open = None
                    return True

                def tick(self, tick, budget=2):
                    n = 0
                    while True:
                        urgent = (
                            self.open is not None and self.open_deadline <= tick + 1
                        ) or (
                            self.open is None
                            and self.jobs
                            and self.jobs[0][0] <= tick + 1
                            and self.jobs[0][1] <= tick
                        )
                        if not urgent and n >= budget:
                            break
                        if not self._pull_one(tick):
                            break
                        n += 1

                def drain(self):
                    while self._pull_one(1 << 30):
                        pass

            filler = Filler()

            # ---------------- attention
            class AttnBlock:
                """Heads A=2g, B=2g+1; query half qh (1024 queries).

                scoresT/exp are ACT-paced.  V matmuls run a few kt behind
                (pending FIFO in the driver) so both heads' exp tiles are
                ready together, letting adjacently issued matmuls with
                disjoint array col groups (V: 0-1 vs 2-3) run concurrently
                on the PE.  vt accumulates A in partitions 0-63 and B in
                64-127 of one bank (memset + start=False keeps the
                interleaved accumulation groups from clearing each other).
                The softmax denominators are accumulated OFF the PE: per-kt
                elementwise adds of the exp tiles into a [128,1024] bf16
                accumulator (GpSimd for head A, DVE for head B), reduced
                across partitions by a single 4-matmul quad at block end.
                Normalization runs entirely off the critical path.
                """

                def __init__(self, g, qh):
                    self.g, self.qoff = g, qh * 1024
                    self.vt = [
                        accp.tile([P, 512], F32, tag=f"vt{qb}", name=f"vt{qb}")
                        for qb in range(2)
                    ]
                    self.cs = accp.tile([P, 512], F32, tag="cs")
                    for t in self.vt:
                        nc.vector.memset(t[:], 0.0)
                    nc.vector.memset(self.cs[:], 0.0)
                    self.ets = {}

                def emit_scores_exp(self, kt):
                    g, qoff = self.g, self.qoff
                    # qb-outer, hp-inner: the two heads' matmuls are emitted
                    # adjacently so their row-disjoint array tiles (rows 0-63
                    # vs 64-127) execute concurrently on the PE
                    ps_s = {
                        hp: psp.tile([P, 1024], F32, tag="ps", name=f"ps_s{hp}")
                        for hp in (0, 1)
                    }
                    for qb in range(2):
                        for hp, pb in ((0, 0), (1, 64)):
                            nc.tensor.matmul(
                                ps_s[hp][:, qb * 512 : (qb + 1) * 512],
                                lhsT=KT[g][pb : pb + 64, kt * P : (kt + 1) * P],
                                rhs=QT[g][
                                    pb : pb + 64,
                                    qoff + qb * 512 : qoff + (qb + 1) * 512,
                                ],
                                start=True,
                                stop=True,
                            )
                    for hp in (0, 1):
                        et = expp.tile([P, 1024], BF16, tag="expT", name=f"et{hp}")
                        nc.scalar.activation(
                            et[:], ps_s[hp][:], mybir.ActivationFunctionType.Exp
                        )
                        self.ets[(kt, hp)] = et

                def emit_v(self, kt):
                    g = self.g
                    last = kt == NKT - 1
                    et = self.ets[kt] = {
                        hp: self.ets.pop((kt, hp)) for hp in (0, 1)
                    }
                    for qb in range(2):
                        for hp, pb in ((0, 0), (1, 64)):
                            nc.tensor.matmul(
                                self.vt[qb][pb : pb + 64, :],
                                lhsT=V_st[kt][:, 2 * g + hp, 0:DK],
                                rhs=et[hp][:, qb * 512 : (qb + 1) * 512],
                                start=False,
                                stop=last,
                                skip_group_check=True,
                                tile_position=(0, pb),
                            )

                def emit_cs(self, kt):
                    last = kt == NKT - 1
                    et = self.ets.pop(kt)
                    for hp in (0, 1):
                        for qb in range(2):
                            cp = 64 * hp + 32 * qb
                            nc.tensor.matmul(
                                self.cs[cp : cp + 1, :],
                                lhsT=ones_bf[:],
                                rhs=et[hp][:, qb * 512 : (qb + 1) * 512],
                                start=False,
                                stop=last,
                                skip_group_check=True,
                                tile_position=(0, cp),
                            )
                    if last:
                        self.emit_norm()

                def emit_norm(self):
                    g, qoff = self.g, self.qoff
                    un = [
                        rcp.tile([P, 512], F32, tag=f"un{qb}", name=f"un{qb}")
                        for qb in range(2)
                    ]
                    for qb in range(2):
                        nc.vector.tensor_copy(out=un[qb][:], in_=self.vt[qb][:])
                    cs_sb = rcp.tile([P, 512], F32, tag="cs_sb")
                    nc.vector.tensor_copy(out=cs_sb[:], in_=self.cs[:])
                    zd = dramp.tile([4, 512], F32, name="zd")
                    # zd rows: 0=(A,qb0) 1=(A,qb1) 2=(B,qb0) 3=(B,qb1)
                    nc.sync.dma_start(zd[:], cs_sb[0:128:32, :])
                    # reciprocal on a [128,16] reshape of the 2048 real Z
                    # values (vs [128,512]: DVE reciprocal is ~8 cyc/col)
                    zs = rcp.tile([P, 16], F32, tag="zs")
                    nc.sync.dma_start(
                        zs[:], zd.rearrange("a (b c) -> (a b) c", c=16)
                    )
                    zr = rcp.tile([P, 16], F32, tag="zr")
                    nc.vector.reciprocal(zr[:], zs[:])
                    zd2 = dramp.tile([4, 512], F32, name="zd2")
                    nc.sync.dma_start(
                        zd2.rearrange("a (b c) -> (a b) c", c=16), zr[:]
                    )
                    for qb in range(2):
                        rcb = rcp.tile(
                            [P, 512], F32, tag=f"rcb{qb}", name=f"rcb{qb}"
                        )
                        nc.sync.dma_start(
                            rcb[0:64, :], zd2[qb, None, :].to_broadcast([64, 512])
                        )
                        nc.sync.dma_start(
                            rcb[64:128, :],
                            zd2[qb + 2, None, :].to_broadcast([64, 512]),
                        )
                        for pb in (0, 64):
                            nc.vector.tensor_mul(
                                out=attn[g][
                                    pb : pb + 64,
                                    qoff + qb * 512 : qoff + (qb + 1) * 512,
                                ],
                                in0=un[qb][pb : pb + 64, :],
                                in1=rcb[pb : pb + 64, :],
                            )

            # ---------------- upfront projections: just enough for the first
            # scores matmul (K0 cols 0-511, Q0 cols 0-1023)
            for job in (
                qk_job(KT[0], w_sb["wk"], 0, 0),
                qk_job(QT[0], w_sb["wq"], 0, 0),
                qk_job(QT[0], w_sb["wq"], 0, 1),
            ):
                for _ in job:
                    pass

            # ---------------- filler job schedule (deadlines in global ticks;
            # 1 tick = one (block, kt) step = ~2.2us of exp pacing).
            # Block order: qh-outer -> block index B = qh*4 + g.
            # V_st[st] feeds emit_v_cs at tick st + LAG of block 0.
            # K[g][sc] feeds scores of block g (qh0) at kt=4*sc.
            # Q[g][qh sc] feeds block qh*4+g from its start.
            jobs = []
            # V_st[st] is consumed by the block-0 v_cs pop at pop_tick[st];
            # derive that from the same lag schedule the driver uses so the
            # projection is always fully emitted before its consumer.
            def lag_at(t):
                if t < 8:
                    return 7
                if t < 13:
                    return 14 - t
                return 1 if t % 2 else 2

            pop_tick = {}
            sim_pending, t = [], 0
            while len(pop_tick) < NKT:
                while len(sim_pending) > lag_at(t):
                    pop_tick[sim_pending.pop(0)] = t
                sim_pending.append(t)
                t += 1
            for st in range(NKT):
                jobs.append((pop_tick[st] - 1, max(0, st // 4), v_job(st)))
            for sc in range(1, NSC):
                jobs.append((4 * sc, sc, qk_job(KT[0], w_sb["wk"], 0, sc)))
            for g in range(1, NG):
                for sc in range(NSC):
                    jobs.append((16 * g + 4 * sc, sc + 1, qk_job(KT[g], w_sb["wk"], g, sc)))
                for sc in range(2):
                    jobs.append((16 * g, sc + 1, qk_job(QT[g], w_sb["wq"], g, sc)))
            for g in range(NG):
                for sc in range(2, NSC):
                    jobs.append((16 * (4 + g), sc + 1, qk_job(QT[g], w_sb["wq"], g, sc)))
            # output projection for the first query half: ready once all qh0
            # blocks are normed (~2 ticks after qh0 ends); no hard deadline.
            WO_INJECT = True
            if WO_INJECT:
                for st in range(8):
                    for ob in range(2):
                        jobs.append((1 << 29, 16 * 4 + 3, wo_job(st, ob)))
            jobs.sort(key=lambda j: j[0])
            for d, r, gen in jobs:
                filler.add(d, r, gen)

            # ---------------- attention driver: pending FIFO carries the
            # V/colsum matmuls LAG ticks behind their exps (deep in block 0 to
            # spread the V-projection burst, paired afterwards so the V and
            # colsum matmul groups batch two kt at a time -- every PE weight-
            # geometry switch costs a ~160ns pipeline-drain bubble, so fewer,
            # larger same-shape groups waste less).  Fillers likewise burst on
            # odd ticks only (urgent deadline work still goes out every tick).
            def emit_batch(popped):
                for b, k in popped:
                    b.emit_v(k)
                for b, k in popped:
                    b.emit_cs(k)

            pending = []
            tick = 0
            for qh in range(2):
                for g in range(NG):
                    blk = AttnBlock(g, qh)
                    for kt in range(NKT):
                        blk.emit_scores_exp(kt)
                        lag = lag_at(tick)
                        popped = []
                        while len(pending) > lag:
                            popped.append(pending.pop(0))
                        emit_batch(popped)
                        pending.append((blk, kt))
                        filler.tick(tick, budget=3 if tick % 2 else 0)
                        tick += 1
            while pending:
                emit_batch(pending[:2])
                del pending[:2]
            filler.drain()

            # ---------------- output projection, second query half
            for st in (range(NKT) if not WO_INJECT else range(8, NKT)):
                for ob in range(2):
                    for _ in wo_job(st, ob):
                        pass

    _split_sync_waits(nc)
    return nc


_NC = None


def _get_nc():
    global _NC
    if _NC is None:
        _NC = build_nc()
    return _NC


# ---------------------------------------------------------------- host side
def make_in_maps(x, wq, wk, wv, wo):
    x = np.asarray(x, dtype=np.float32)
    wq = np.asarray(wq, dtype=np.float32)
    wk = np.asarray(wk, dtype=np.float32)
    wv = np.asarray(wv, dtype=np.float32)
    wo = np.asarray(wo, dtype=np.float32)
    in_maps = []
    for c in range(N_CORES):
        b, hg = c // 2, c % 2
        sl = slice(hg * DL, (hg + 1) * DL)
        xTc = np.ascontiguousarray(x[b].T).astype(BF16_NP)
        wqTc = np.ascontiguousarray((wq[sl] / 8.0).T).astype(BF16_NP)
        wkTc = np.ascontiguousarray(wk[sl].T).astype(BF16_NP)
        wvTc = np.ascontiguousarray(wv[sl].T).astype(BF16_NP)
        woTc = np.ascontiguousarray(wo[:, sl].T).astype(BF16_NP)
        in_maps.append(
            {"xT": xTc, "wqT": wqTc, "wkT": wkTc, "wvT": wvTc, "woT": woTc}
        )
    return in_maps


def gather(results):
    out = np.zeros((4, S, DM), dtype=np.float32)
    for c in range(N_CORES):
        out[c // 2] += results[c]["out"]
    return out


def kernel(x, wq, wk, wv, wo):
    from concourse.bass_utils import run_bass_kernel_spmd

    nc = _get_nc()
    in_maps = make_in_maps(x, wq, wk, wv, wo)
    res = run_bass_kernel_spmd(nc, in_maps, CORE_IDS)
    return gather(res.results)


# revision 22
# speedup vs baseline: 1.2008x; 1.0010x over previous
"""Multi-head self-attention (B=4, S=2048, D=1024, H=16) on 8 trn2 NeuronCores.

Sharding: batch (4) x head-group (2 groups of 8 heads) -> 8 cores.
Each core computes, for its (batch b, head-group hg):
  Q'^T = (wq_l/8) @ x_b^T            [512, 2048]   (1/sqrt(dk) folded into wq)
  K^T  = wk_l @ x_b^T                [512, 2048]
  V    = x_b @ wv_l^T                [2048, 512]
  per head h (8 local, dk=64), in transposed layout (keys on partitions):
    scoresT[k, q] = K_h @ Q'_h^T     (no max-subtraction: scores ~ N(0,4), exp
                                      of |s|<~12 is safe in fp32/bf16)
    expT = exp(scoresT)              (ScalarE, PSUM->SBUF bf16)
    unnormT[c, q] = V_h^T @ expT     (PE, accumulated over key tiles)
    Z[q] = ones^T @ expT             (PE colsum quads, same accumulation)
    attnT = unnormT / Z              (reciprocal once + DMA partition
                                      broadcast via DRAM + DVE muls)
  out_partial = attnT^T @ wo_l^T     [2048, 1024]  (row-parallel wo)
Host sums the two partials per batch (the "all-reduce" of row-parallel wo).

v2 schedule: the 256 exps (ScalarE, ~1.11us each) are the pacer.  Blocks run
qh-outer (all 4 head-pairs for queries 0-1023, then 1024-2047) so the output
projection for the first query half can overlap the second attention phase.
All projection work except the first three Q/K chunks is deferred into the
attention stream as fine-grained (2-matmul) filler pieces pulled by a
deadline-driven queue, so the PE never blocks the exp chain for more than
~0.5us at a time.  DMA is staged (wk+xq0 first, then wq+wv, then the rest) so
the first scores matmul can issue at ~15us.
"""

import ml_dtypes
import numpy as np

import bass_rust
import concourse.bass as bass
import concourse.mybir as mybir
import concourse.tile as tile

# ---------------------------------------------------------------- constants
S = 2048          # sequence length
DM = 1024         # model dim
DL = 512          # local (per-core) head dims = 8 heads * 64
DK = 64           # head dim
P = 128
NKT = S // P      # 16 key tiles
NG = DL // P      # 4 head-pairs (c-tiles / dq-tiles)
KD = DM // P      # 8 contraction tiles for projections
NSC = S // 512    # 4 s-chunks for projections
F32 = mybir.dt.float32
BF16 = mybir.dt.bfloat16
BF16_NP = ml_dtypes.bfloat16

N_CORES = 8
CORE_IDS = list(range(N_CORES))


# ------------------------------------------------- walrus sync-wait workaround
def _split_sync_waits(nc, limit=1):
    """This toolchain's walrus codegen rejects instructions carrying more than
    one sync-wait command.  Move excess waits onto dedicated same-engine nops
    inserted immediately before the instruction (sequential waits on the same
    engine queue are semantically identical to multiple waits on one inst)."""
    fn = nc.m.functions[0]
    snapshots = [(bb, list(bb.instructions)) for bb in fn.blocks]
    plans = []
    for _bb, insts in snapshots:
        plan = {}
        for idx, inst in enumerate(insts):
            si = inst.sync_info
            waits = list(si.on_wait) if si and si.on_wait else []
            if len(waits) > limit:
                pre, keep = waits[:-limit], waits[-limit:]
                nops = []
                for w in pre:
                    ni = nc.engines[inst.engine].nop(nofuse=True, hint="wsplit").ins
                    ni.sync_info = bass_rust.SyncInfo(on_wait=[w], on_update=[])
                    nops.append(ni)
                si.on_wait = keep
                plan[idx] = nops
        plans.append(plan)
    # Rebuild every block from its pre-pass snapshot plus insertions; this also
    # drops the fresh nops from wherever bass appended them at creation time.
    for (bb, insts), plan in zip(snapshots, plans):
        out = []
        for idx, inst in enumerate(insts):
            out.extend(plan.get(idx, ()))
            out.append(inst)
        bb.instructions = out


# ---------------------------------------------------------------- the program
def build_nc():
    """Build the SPMD per-core Bass program (identical on all 8 cores)."""
    nc = bass.Bass()

    xT = nc.declare_dram_parameter("xT", [DM, S], BF16, isOutput=False)
    wqT = nc.declare_dram_parameter("wqT", [DM, DL], BF16, isOutput=False)
    wkT = nc.declare_dram_parameter("wkT", [DM, DL], BF16, isOutput=False)
    wvT = nc.declare_dram_parameter("wvT", [DM, DL], BF16, isOutput=False)
    woT = nc.declare_dram_parameter("woT", [DL, DM], BF16, isOutput=False)
    out = nc.declare_dram_parameter("out", [S, DM], F32, isOutput=True)

    with tile.TileContext(nc) as tc:
        with (
            tc.tile_pool(name="big", bufs=1) as big,
            tc.tile_pool(name="expT", bufs=16) as expp,
            tc.tile_pool(name="rc", bufs=2) as rcp,
            tc.tile_pool(name="outsb", bufs=3) as outp,
            tc.tile_pool(name="dram", bufs=2, space="DRAM") as dramp,
            tc.tile_pool(name="ps", bufs=2, space="PSUM") as psp,
            tc.tile_pool(name="acc", bufs=1, space="PSUM") as accp,
            tc.tile_pool(name="fil", bufs=1, space="PSUM") as filp,
        ):
            # ---------------- staged DRAM loads.  Stage 1 (wk + first x
            # quarter) ships alone so the first K/Q projection chunks can
            # start at ~15us; later stages are gated behind earlier tensors
            # via 1-element gpsimd copies (RAW on the gating tile, WAW on the
            # staged destination) so they don't steal HBM bandwidth early.
            w_sb = {}
            for name in ("wk", "wq", "wv"):
                w_sb[name] = big.tile([P, KD, DL], BF16, tag=name, name=name)
            xT_r = xT.rearrange("(kd p) s -> p kd s", p=P)
            xT_q = [
                big.tile([P, KD, 512], BF16, tag=f"xT{j}", name=f"xTq{j}")
                for j in range(4)
            ]
            woT_sb = big.tile([P, NG, DM], BF16, tag="wo")

            def stage(dsts, gate):
                if gate is not None:
                    for d in dsts:
                        nc.gpsimd.tensor_copy(
                            out=d[0:1, 0, 0:1], in_=gate[0:1, 0, 0:1]
                        )
            # stage 1: wk, wq, xq0 (feeds the upfront K0/Q0 chunks)
            stage([w_sb["wk"], w_sb["wq"], xT_q[0]], None)
            nc.sync.dma_start(
                w_sb["wk"][:], wkT.rearrange("(kd p) m -> p kd m", p=P)
            )
            nc.sync.dma_start(
                w_sb["wq"][:], wqT.rearrange("(kd p) m -> p kd m", p=P)
            )
            nc.sync.dma_start(xT_q[0][:], xT_r[:, :, 0:512])
            # stage 2: wv + xq1 (gated on wk): Q0 sc1 reads x cols 512-1023
            stage([w_sb["wv"], xT_q[1]], w_sb["wk"])
            nc.sync.dma_start(
                w_sb["wv"][:], wvT.rearrange("(kd p) m -> p kd m", p=P)
            )
            nc.sync.dma_start(xT_q[1][:], xT_r[:, :, 512:1024])
            # stage 3: xq2, then xq3+wo chained behind
            stage([xT_q[2]], xT_q[1])
            nc.sync.dma_start(xT_q[2][:], xT_r[:, :, 1024:1536])
            stage([xT_q[3], woT_sb], xT_q[2])
            nc.sync.dma_start(xT_q[3][:], xT_r[:, :, 1536:2048])
            nc.sync.dma_start(woT_sb[:], woT.rearrange("(ct p) o -> p ct o", p=P))

            def xslice(kd, fr, to):
                q = fr // 512
                assert to <= (q + 1) * 512
                return xT_q[q][:, kd, fr - q * 512 : to - q * 512]

            # ---------------- constants
            ones_bf = big.tile([P, 1], BF16, tag="ones")
            nc.vector.memset(ones_bf[:], 1.0)
            warm = big.tile([P, 512], BF16, tag="warm")
            nc.vector.memset(warm[:], 0.0)

            # HAM warm-up: keep the PE busy on throwaway matmuls while the
            # stage-1 DMA lands so the first real chunks run at 2.4GHz.
            ps_w = psp.tile([P, 512], F32, tag="ps", name="warmps")
            for _ in range(44):
                nc.tensor.matmul(
                    ps_w[:], lhsT=warm[:, 0:128], rhs=warm[:], start=True,
                    stop=True,
                )

            # persistent activation tensors
            QT = [big.tile([P, S], BF16, tag=f"QT{g}", name=f"QT{g}") for g in range(NG)]
            KT = [big.tile([P, S], BF16, tag=f"KT{g}", name=f"KT{g}") for g in range(NG)]
            V_st = [big.tile([P, 8, DK + 1], BF16, tag=f"V{st}", name=f"V{st}") for st in range(NKT)]
            attn = [big.tile([P, S], BF16, tag=f"attn{g}", name=f"attn{g}") for g in range(NG)]

            # ---------------- filler machinery: all projection / output work
            # is expressed as jobs that yield 2-matmul pieces; the attention
            # driver pulls pieces by deadline so the exp chain never waits
            # long on the PE queue.
            def qk_job(dst, w, g, sc):
                ps = filp.tile([P, 512], F32, tag="fil", name="projch")
                for kd0 in range(0, KD, 4):
                    for kd in range(kd0, kd0 + 4):
                        nc.tensor.matmul(
                            ps[:],
                            lhsT=w[:, kd, g * P : (g + 1) * P],
                            rhs=xslice(kd, sc * 512, (sc + 1) * 512),
                            start=(kd == 0),
                            stop=(kd == KD - 1),
                        )
                    yield
                nc.vector.tensor_copy(
                    out=dst[:, sc * 512 : (sc + 1) * 512], in_=ps[:]
                )

            def v_job(st):
                ps = filp.tile([P, 512], F32, tag="fil", name="vch")
                for kd0 in range(0, KD, 4):
                    for kd in range(kd0, kd0 + 4):
                        nc.tensor.matmul(
                            ps[:],
                            lhsT=xslice(kd, st * P, (st + 1) * P),
                            rhs=w_sb["wv"][:, kd, :],
                            start=(kd == 0),
                            stop=(kd == KD - 1),
                        )
                    yield
                nc.vector.tensor_copy(
                    out=V_st[st][:, :, 0:DK],
                    in_=ps.rearrange("p (h c) -> p h c", c=DK),
                )

            def wo_job(st, ob):
                ps = filp.tile([P, 512], F32, tag="fil", name="wochunk")
                for ct0 in (0, 2):
                    for ct in (ct0, ct0 + 1):
                        nc.tensor.matmul(
                            ps[:],
                            lhsT=attn[ct][:, st * P : (st + 1) * P],
                            rhs=woT_sb[:, ct, ob * 512 : (ob + 1) * 512],
                            start=(ct == 0),
                            stop=(ct == NG - 1),
                        )
                    yield
                ot = outp.tile([P, 512], F32, tag="out")
                nc.vector.tensor_copy(out=ot[:], in_=ps[:])
                nc.sync.dma_start(
                    out[st * P : (st + 1) * P, ob * 512 : (ob + 1) * 512], ot[:]
                )

            class Filler:
                """Ordered queue of (deadline_tick, ready_tick, job-generator).
                One job open at a time (so filler holds a single av psum
                slot); pieces are pulled per tick: everything past deadline
                unconditionally, plus up to `budget` opportunistic pieces."""

                def __init__(self):
                    self.jobs = []
                    self.open = None
                    self.open_deadline = 0

                def add(self, deadline, ready, gen):
                    self.jobs.append((deadline, ready, gen))

                def _pull_one(self, tick):
                    if self.open is None:
                        if not self.jobs or self.jobs[0][1] > tick:
                            return False
                        self.open_deadline, _, self.open = self.jobs.pop(0)
                    try:
                        next(self.open)
                    except StopIteration:
                        self.open = None
                    return True

                def tick(self, tick, budget=2):
                    n = 0
                    while True:
                        urgent = (
                            self.open is not None and self.open_deadline <= tick + 1
                        ) or (
                            self.open is None
                            and self.jobs
                            and self.jobs[0][0] <= tick + 1
                            and self.jobs[0][1] <= tick
                        )
                        if not urgent and n >= budget:
                            break
                        if not self._pull_one(tick):
                            break
                        n += 1

                def drain(self):
                    while self._pull_one(1 << 30):
                        pass

            filler = Filler()

            # ---------------- attention
            class AttnBlock:
                """Heads A=2g, B=2g+1; query half qh (1024 queries).

                scoresT/exp are ACT-paced.  V matmuls run a few kt behind
                (pending FIFO in the driver) so both heads' exp tiles are
                ready together, letting adjacently issued matmuls with
                disjoint array col groups (V: 0-1 vs 2-3) run concurrently
                on the PE.  vt accumulates A in partitions 0-63 and B in
                64-127 of one bank (memset + start=False keeps the
                interleaved accumulation groups from clearing each other).
                The softmax denominators are accumulated OFF the PE: per-kt
                elementwise adds of the exp tiles into a [128,1024] bf16
                accumulator (GpSimd for head A, DVE for head B), reduced
                across partitions by a single 4-matmul quad at block end.
                Normalization runs entirely off the critical path.
                """

                def __init__(self, g, qh):
                    self.g, self.qoff = g, qh * 1024
                    self.vt = [
                        accp.tile([P, 512], F32, tag=f"vt{qb}", name=f"vt{qb}")
                        for qb in range(2)
                    ]
                    self.cs = accp.tile([P, 512], F32, tag="cs")
                    for t in self.vt:
                        nc.vector.memset(t[:], 0.0)
                    nc.vector.memset(self.cs[:], 0.0)
                    self.ets = {}

                def emit_scores_exp(self, kt):
                    g, qoff = self.g, self.qoff
                    # qb-outer, hp-inner: the two heads' matmuls are emitted
                    # adjacently so their row-disjoint array tiles (rows 0-63
                    # vs 64-127) execute concurrently on the PE
                    ps_s = {
                        hp: psp.tile([P, 1024], F32, tag="ps", name=f"ps_s{hp}")
                        for hp in (0, 1)
                    }
                    for qb in range(2):
                        for hp, pb in ((0, 0), (1, 64)):
                            nc.tensor.matmul(
                                ps_s[hp][:, qb * 512 : (qb + 1) * 512],
                                lhsT=KT[g][pb : pb + 64, kt * P : (kt + 1) * P],
                                rhs=QT[g][
                                    pb : pb + 64,
                                    qoff + qb * 512 : qoff + (qb + 1) * 512,
                                ],
                                start=True,
                                stop=True,
                            )
                    for hp in (0, 1):
                        et = expp.tile([P, 1024], BF16, tag="expT", name=f"et{hp}")
                        nc.scalar.activation(
                            et[:], ps_s[hp][:], mybir.ActivationFunctionType.Exp
                        )
                        self.ets[(kt, hp)] = et

                def emit_v(self, kt):
                    g = self.g
                    last = kt == NKT - 1
                    et = self.ets[kt] = {
                        hp: self.ets.pop((kt, hp)) for hp in (0, 1)
                    }
                    for qb in range(2):
                        for hp, pb in ((0, 0), (1, 64)):
                            nc.tensor.matmul(
                                self.vt[qb][pb : pb + 64, :],
                                lhsT=V_st[kt][:, 2 * g + hp, 0:DK],
                                rhs=et[hp][:, qb * 512 : (qb + 1) * 512],
                                start=False,
                                stop=last,
                                skip_group_check=True,
                                tile_position=(0, pb),
                            )

                def emit_cs(self, kt):
                    last = kt == NKT - 1
                    et = self.ets.pop(kt)
                    for hp in (0, 1):
                        for qb in range(2):
                            cp = 64 * hp + 32 * qb
                            nc.tensor.matmul(
                                self.cs[cp : cp + 1, :],
                                lhsT=ones_bf[:],
                                rhs=et[hp][:, qb * 512 : (qb + 1) * 512],
                                start=False,
                                stop=last,
                                skip_group_check=True,
                                tile_position=(0, cp),
                            )
                    if last:
                        self.emit_norm()

                def emit_norm(self):
                    g, qoff = self.g, self.qoff
                    un = [
                        rcp.tile([P, 512], F32, tag=f"un{qb}", name=f"un{qb}")
                        for qb in range(2)
                    ]
                    for qb in range(2):
                        nc.vector.tensor_copy(out=un[qb][:], in_=self.vt[qb][:])
                    cs_sb = rcp.tile([P, 512], F32, tag="cs_sb")
                    nc.vector.tensor_copy(out=cs_sb[:], in_=self.cs[:])
                    zd = dramp.tile([4, 512], F32, name="zd")
                    # zd rows: 0=(A,qb0) 1=(A,qb1) 2=(B,qb0) 3=(B,qb1)
                    nc.sync.dma_start(zd[:], cs_sb[0:128:32, :])
                    # reciprocal on a [128,16] reshape of the 2048 real Z
                    # values (vs [128,512]: DVE reciprocal is ~8 cyc/col)
                    zs = rcp.tile([P, 16], F32, tag="zs")
                    nc.sync.dma_start(
                        zs[:], zd.rearrange("a (b c) -> (a b) c", c=16)
                    )
                    zr = rcp.tile([P, 16], F32, tag="zr")
                    nc.vector.reciprocal(zr[:], zs[:])
                    zd2 = dramp.tile([4, 512], F32, name="zd2")
                    nc.sync.dma_start(
                        zd2.rearrange("a (b c) -> (a b) c", c=16), zr[:]
                    )
                    for qb in range(2):
                        rcb = rcp.tile(
                            [P, 512], F32, tag=f"rcb{qb}", name=f"rcb{qb}"
                        )
                        nc.sync.dma_start(
                            rcb[0:64, :], zd2[qb, None, :].to_broadcast([64, 512])
                        )
                        nc.sync.dma_start(
                            rcb[64:128, :],
                            zd2[qb + 2, None, :].to_broadcast([64, 512]),
                        )
                        for pb in (0, 64):
                            nc.vector.tensor_mul(
                                out=attn[g][
                                    pb : pb + 64,
                                    qoff + qb * 512 : qoff + (qb + 1) * 512,
                                ],
                                in0=un[qb][pb : pb + 64, :],
                                in1=rcb[pb : pb + 64, :],
                            )

            # ---------------- upfront projections: just enough for the first
            # scores matmul (K0 cols 0-511, Q0 cols 0-1023)
            for job in (
                qk_job(KT[0], w_sb["wk"], 0, 0),
                qk_job(QT[0], w_sb["wq"], 0, 0),
                qk_job(QT[0], w_sb["wq"], 0, 1),
            ):
                for _ in job:
                    pass

            # ---------------- filler job schedule (deadlines in global ticks;
            # 1 tick = one (block, kt) step = ~2.2us of exp pacing).
            # Block order: qh-outer -> block index B = qh*4 + g.
            # V_st[st] feeds emit_v_cs at tick st + LAG of block 0.
            # K[g][sc] feeds scores of block g (qh0) at kt=4*sc.
            # Q[g][qh sc] feeds block qh*4+g from its start.
            jobs = []
            # V_st[st] is consumed by the block-0 v_cs pop at pop_tick[st];
            # derive that from the same lag schedule the driver uses so the
            # projection is always fully emitted before its consumer.
            def lag_at(t):
                if t < 8:
                    return 7
                if t < 13:
                    return 14 - t
                return 1 if t % 2 else 2

            pop_tick = {}
            sim_pending, t = [], 0
            while len(pop_tick) < NKT:
                while len(sim_pending) > lag_at(t):
                    pop_tick[sim_pending.pop(0)] = t
                sim_pending.append(t)
                t += 1
            for st in range(NKT):
                jobs.append((pop_tick[st] - 1, max(0, st // 4), v_job(st)))
            for sc in range(1, NSC):
                jobs.append((4 * sc, sc, qk_job(KT[0], w_sb["wk"], 0, sc)))
            def lead(d):
                return d - 8 if d >= 32 else d
            for g in range(1, NG):
                for sc in range(NSC):
                    jobs.append((lead(16 * g + 4 * sc), sc + 1, qk_job(KT[g], w_sb["wk"], g, sc)))
                for sc in range(2):
                    jobs.append((lead(16 * g), sc + 1, qk_job(QT[g], w_sb["wq"], g, sc)))
            for g in range(NG):
                for sc in range(2, NSC):
                    jobs.append((lead(16 * (4 + g)) - 4, sc + 1, qk_job(QT[g], w_sb["wq"], g, sc)))
            # output projection for the first query half: ready once all qh0
            # blocks are normed (~2 ticks after qh0 ends); no hard deadline.
            WO_INJECT = True
            if WO_INJECT:
                for st in range(5):
                    for ob in range(2):
                        jobs.append((1 << 29, 16 * 4 + 3, wo_job(st, ob)))
            jobs.sort(key=lambda j: j[0])
            for d, r, gen in jobs:
                filler.add(d, r, gen)

            # ---------------- attention driver: pending FIFO carries the
            # V/colsum matmuls LAG ticks behind their exps (deep in block 0 to
            # spread the V-projection burst, paired afterwards so the V and
            # colsum matmul groups batch two kt at a time -- every PE weight-
            # geometry switch costs a ~160ns pipeline-drain bubble, so fewer,
            # larger same-shape groups waste less).  Fillers likewise burst on
            # odd ticks only (urgent deadline work still goes out every tick).
            def emit_batch(popped):
                for b, k in popped:
                    b.emit_v(k)
                for b, k in popped:
                    b.emit_cs(k)

            pending = []
            tick = 0
            for qh in range(2):
                for g in range(NG):
                    blk = AttnBlock(g, qh)
                    for kt in range(NKT):
                        blk.emit_scores_exp(kt)
                        lag = lag_at(tick)
                        popped = []
                        while len(pending) > lag:
                            popped.append(pending.pop(0))
                        emit_batch(popped)
                        pending.append((blk, kt))
                        filler.tick(tick, budget=3 if tick % 2 else 0)
                        tick += 1
            while pending:
                emit_batch(pending[:2])
                del pending[:2]
            filler.drain()

            # ---------------- output projection tail.  The held-back qh0
            # chunks (st 5-7) go first: their inputs are long ready, so they
            # keep the PE busy (and the HAM clock-gate warm) while the last
            # block's normalization pipeline drains.
            tail_sts = ([5, 6, 7] if WO_INJECT else list(range(8))) + list(
                range(8, NKT)
            )
            for st in tail_sts:
                for ob in range(2):
                    for _ in wo_job(st, ob):
                        pass

    _split_sync_waits(nc)
    return nc


_NC = None


def _get_nc():
    global _NC
    if _NC is None:
        _NC = build_nc()
    return _NC


# ---------------------------------------------------------------- host side
def make_in_maps(x, wq, wk, wv, wo):
    x = np.asarray(x, dtype=np.float32)
    wq = np.asarray(wq, dtype=np.float32)
    wk = np.asarray(wk, dtype=np.float32)
    wv = np.asarray(wv, dtype=np.float32)
    wo = np.asarray(wo, dtype=np.float32)
    in_maps = []
    for c in range(N_CORES):
        b, hg = c // 2, c % 2
        sl = slice(hg * DL, (hg + 1) * DL)
        xTc = np.ascontiguousarray(x[b].T).astype(BF16_NP)
        wqTc = np.ascontiguousarray((wq[sl] / 8.0).T).astype(BF16_NP)
        wkTc = np.ascontiguousarray(wk[sl].T).astype(BF16_NP)
        wvTc = np.ascontiguousarray(wv[sl].T).astype(BF16_NP)
        woTc = np.ascontiguousarray(wo[:, sl].T).astype(BF16_NP)
        in_maps.append(
            {"xT": xTc, "wqT": wqTc, "wkT": wkTc, "wvT": wvTc, "woT": woTc}
        )
    return in_maps


def gather(results):
    out = np.zeros((4, S, DM), dtype=np.float32)
    for c in range(N_CORES):
        out[c // 2] += results[c]["out"]
    return out


def kernel(x, wq, wk, wv, wo):
    from concourse.bass_utils import run_bass_kernel_spmd

    nc = _get_nc()
    in_maps = make_in_maps(x, wq, wk, wv, wo)
    res = run_bass_kernel_spmd(nc, in_maps, CORE_IDS)
    return gather(res.results)


# revision 23
# speedup vs baseline: 1.2872x; 1.0720x over previous
"""Multi-head self-attention (B=4, S=2048, D=1024, H=16) on 8 trn2 NeuronCores.

Sharding: batch (4) x head-group (2 groups of 8 heads) -> 8 cores.
Each core computes, for its (batch b, head-group hg):
  Q'^T = (wq_l/8) @ x_b^T            [512, 2048]   (1/sqrt(dk) folded into wq)
  K^T  = wk_l @ x_b^T                [512, 2048]
  V    = x_b @ wv_l^T                [2048, 512]
  per head h (8 local, dk=64), in transposed layout (keys on partitions):
    scoresT[k, q] = K_h @ Q'_h^T     (no max-subtraction: scores ~ N(0,4), exp
                                      of |s|<~12 is safe in fp32/bf16)
    expT = exp(scoresT)              (ScalarE, PSUM->SBUF bf16)
    unnormT[c, q] = V_h^T @ expT     (PE, accumulated over key tiles)
    Z[q] = ones^T @ expT             (PE colsum quads, same accumulation)
    attnT = unnormT / Z              (reciprocal once + DMA partition
                                      broadcast via DRAM + DVE muls)
  out_partial = attnT^T @ wo_l^T     [2048, 1024]  (row-parallel wo)
Host sums the two partials per batch (the "all-reduce" of row-parallel wo).

v2 schedule: the 256 exps (ScalarE, ~1.11us each) are the pacer.  Blocks run
qh-outer (all 4 head-pairs for queries 0-1023, then 1024-2047) so the output
projection for the first query half can overlap the second attention phase.
All projection work except the first three Q/K chunks is deferred into the
attention stream as fine-grained (2-matmul) filler pieces pulled by a
deadline-driven queue, so the PE never blocks the exp chain for more than
~0.5us at a time.  DMA is staged (wk+xq0 first, then wq+wv, then the rest) so
the first scores matmul can issue at ~15us.
"""

import ml_dtypes
import numpy as np

import bass_rust
import concourse.bass as bass
import concourse.mybir as mybir
import concourse.tile as tile

# ---------------------------------------------------------------- constants
S = 2048          # sequence length
DM = 1024         # model dim
DL = 512          # local (per-core) head dims = 8 heads * 64
DK = 64           # head dim
P = 128
NKT = S // P      # 16 key tiles
NG = DL // P      # 4 head-pairs (c-tiles / dq-tiles)
KD = DM // P      # 8 contraction tiles for projections
NSC = S // 512    # 4 s-chunks for projections
F32 = mybir.dt.float32
BF16 = mybir.dt.bfloat16
BF16_NP = ml_dtypes.bfloat16

N_CORES = 8
CORE_IDS = list(range(N_CORES))


# ------------------------------------------------- walrus sync-wait workaround
def _split_sync_waits(nc, limit=1):
    """This toolchain's walrus codegen rejects instructions carrying more than
    one sync-wait command.  Move excess waits onto dedicated same-engine nops
    inserted immediately before the instruction (sequential waits on the same
    engine queue are semantically identical to multiple waits on one inst)."""
    fn = nc.m.functions[0]
    snapshots = [(bb, list(bb.instructions)) for bb in fn.blocks]
    plans = []
    for _bb, insts in snapshots:
        plan = {}
        for idx, inst in enumerate(insts):
            si = inst.sync_info
            waits = list(si.on_wait) if si and si.on_wait else []
            if len(waits) > limit:
                pre, keep = waits[:-limit], waits[-limit:]
                nops = []
                for w in pre:
                    ni = nc.engines[inst.engine].nop(nofuse=True, hint="wsplit").ins
                    ni.sync_info = bass_rust.SyncInfo(on_wait=[w], on_update=[])
                    nops.append(ni)
                si.on_wait = keep
                plan[idx] = nops
        plans.append(plan)
    # Rebuild every block from its pre-pass snapshot plus insertions; this also
    # drops the fresh nops from wherever bass appended them at creation time.
    for (bb, insts), plan in zip(snapshots, plans):
        out = []
        for idx, inst in enumerate(insts):
            out.extend(plan.get(idx, ()))
            out.append(inst)
        bb.instructions = out


# ---------------------------------------------------------------- the program
def build_nc():
    """Build the SPMD per-core Bass program (identical on all 8 cores)."""
    nc = bass.Bass()

    xT = nc.declare_dram_parameter("xT", [DM, S], BF16, isOutput=False)
    wqT = nc.declare_dram_parameter("wqT", [DM, DL], BF16, isOutput=False)
    wkT = nc.declare_dram_parameter("wkT", [DM, DL], BF16, isOutput=False)
    wvT = nc.declare_dram_parameter("wvT", [DM, DL], BF16, isOutput=False)
    woT = nc.declare_dram_parameter("woT", [DL, DM], BF16, isOutput=False)
    out = nc.declare_dram_parameter("out", [S, DM], F32, isOutput=True)

    with tile.TileContext(nc) as tc:
        with (
            tc.tile_pool(name="big", bufs=1) as big,
            tc.tile_pool(name="expT", bufs=16) as expp,
            tc.tile_pool(name="rc", bufs=2) as rcp,
            tc.tile_pool(name="outsb", bufs=3) as outp,
            tc.tile_pool(name="dram", bufs=2, space="DRAM") as dramp,
            tc.tile_pool(name="ps", bufs=2, space="PSUM") as psp,
            tc.tile_pool(name="acc", bufs=1, space="PSUM") as accp,
            tc.tile_pool(name="fil", bufs=1, space="PSUM") as filp,
        ):
            # ---------------- staged DRAM loads.  Stage 1 (wk + first x
            # quarter) ships alone so the first K/Q projection chunks can
            # start at ~15us; later stages are gated behind earlier tensors
            # via 1-element gpsimd copies (RAW on the gating tile, WAW on the
            # staged destination) so they don't steal HBM bandwidth early.
            w_sb = {}
            for name in ("wk", "wq", "wv"):
                w_sb[name] = big.tile([P, KD, DL], BF16, tag=name, name=name)
            xT_r = xT.rearrange("(kd p) s -> p kd s", p=P)
            xT_q = [
                big.tile([P, KD, 512], BF16, tag=f"xT{j}", name=f"xTq{j}")
                for j in range(4)
            ]
            woT_sb = big.tile([P, NG, DM], BF16, tag="wo")

            def stage(dsts, gate):
                if gate is not None:
                    for d in dsts:
                        nc.gpsimd.tensor_copy(
                            out=d[0:1, 0, 0:1], in_=gate[0:1, 0, 0:1]
                        )
            # stage 1: wk, wq, xq0 (feeds the upfront K0/Q0 chunks)
            stage([w_sb["wk"], w_sb["wq"], xT_q[0]], None)
            nc.sync.dma_start(
                w_sb["wk"][:], wkT.rearrange("(kd p) m -> p kd m", p=P)
            )
            nc.sync.dma_start(
                w_sb["wq"][:], wqT.rearrange("(kd p) m -> p kd m", p=P)
            )
            nc.sync.dma_start(xT_q[0][:], xT_r[:, :, 0:512])
            # stage 2: wv + xq1 (gated on wk): Q0 sc1 reads x cols 512-1023
            stage([w_sb["wv"], xT_q[1]], w_sb["wk"])
            nc.sync.dma_start(
                w_sb["wv"][:], wvT.rearrange("(kd p) m -> p kd m", p=P)
            )
            nc.sync.dma_start(xT_q[1][:], xT_r[:, :, 512:1024])
            # stage 3: xq2, then xq3+wo chained behind
            stage([xT_q[2]], xT_q[1])
            nc.sync.dma_start(xT_q[2][:], xT_r[:, :, 1024:1536])
            stage([xT_q[3], woT_sb], xT_q[2])
            nc.sync.dma_start(xT_q[3][:], xT_r[:, :, 1536:2048])
            nc.sync.dma_start(woT_sb[:], woT.rearrange("(ct p) o -> p ct o", p=P))

            def xslice(kd, fr, to):
                q = fr // 512
                assert to <= (q + 1) * 512
                return xT_q[q][:, kd, fr - q * 512 : to - q * 512]

            # ---------------- constants
            ones_bf = big.tile([P, 1], BF16, tag="ones")
            nc.vector.memset(ones_bf[:], 1.0)
            warm = big.tile([P, 512], BF16, tag="warm")
            nc.vector.memset(warm[:], 0.0)

            # HAM warm-up: keep the PE busy on throwaway matmuls while the
            # stage-1 DMA lands so the first real chunks run at 2.4GHz.
            ps_w = psp.tile([P, 512], F32, tag="ps", name="warmps")
            for _ in range(44):
                nc.tensor.matmul(
                    ps_w[:], lhsT=warm[:, 0:128], rhs=warm[:], start=True,
                    stop=True,
                )

            # persistent activation tensors
            QT = [big.tile([P, S], BF16, tag=f"QT{g}", name=f"QT{g}") for g in range(NG)]
            KT = [big.tile([P, S], BF16, tag=f"KT{g}", name=f"KT{g}") for g in range(NG)]
            V_st = [big.tile([P, 8, DK + 1], BF16, tag=f"V{st}", name=f"V{st}") for st in range(NKT)]
            attn = [big.tile([P, S], BF16, tag=f"attn{g}", name=f"attn{g}") for g in range(NG)]

            # ---------------- filler machinery: all projection / output work
            # is expressed as jobs that yield 2-matmul pieces; the attention
            # driver pulls pieces by deadline so the exp chain never waits
            # long on the PE queue.
            def qk_job(dst, w, g, sc):
                ps = filp.tile([P, 512], F32, tag="fil", name="projch")
                for kd0 in range(0, KD, 4):
                    for kd in range(kd0, kd0 + 4):
                        nc.tensor.matmul(
                            ps[:],
                            lhsT=w[:, kd, g * P : (g + 1) * P],
                            rhs=xslice(kd, sc * 512, (sc + 1) * 512),
                            start=(kd == 0),
                            stop=(kd == KD - 1),
                        )
                    yield
                nc.vector.tensor_copy(
                    out=dst[:, sc * 512 : (sc + 1) * 512], in_=ps[:]
                )

            def v_job(st):
                ps = filp.tile([P, 512], F32, tag="fil", name="vch")
                for kd0 in range(0, KD, 4):
                    for kd in range(kd0, kd0 + 4):
                        nc.tensor.matmul(
                            ps[:],
                            lhsT=xslice(kd, st * P, (st + 1) * P),
                            rhs=w_sb["wv"][:, kd, :],
                            start=(kd == 0),
                            stop=(kd == KD - 1),
                        )
                    yield
                nc.vector.tensor_copy(
                    out=V_st[st][:, :, 0:DK],
                    in_=ps.rearrange("p (h c) -> p h c", c=DK),
                )

            def wo_job(st, ob, pool=None, tag="fil"):
                ps = (pool or filp).tile([P, 512], F32, tag=tag, name="wochunk")
                for ct0 in (0, 2):
                    for ct in (ct0, ct0 + 1):
                        nc.tensor.matmul(
                            ps[:],
                            lhsT=attn[ct][:, st * P : (st + 1) * P],
                            rhs=woT_sb[:, ct, ob * 512 : (ob + 1) * 512],
                            start=(ct == 0),
                            stop=(ct == NG - 1),
                        )
                    yield
                ot = outp.tile([P, 512], F32, tag="out")
                nc.vector.tensor_copy(out=ot[:], in_=ps[:])
                nc.sync.dma_start(
                    out[st * P : (st + 1) * P, ob * 512 : (ob + 1) * 512], ot[:]
                )

            class Filler:
                """Ordered queue of (deadline_tick, ready_tick, job-generator).
                One job open at a time (so filler holds a single av psum
                slot); pieces are pulled per tick: everything past deadline
                unconditionally, plus up to `budget` opportunistic pieces."""

                def __init__(self):
                    self.jobs = []
                    self.open = None
                    self.open_deadline = 0

                def add(self, deadline, ready, gen):
                    self.jobs.append((deadline, ready, gen))

                def _pull_one(self, tick):
                    if self.open is None:
                        if not self.jobs or self.jobs[0][1] > tick:
                            return False
                        self.open_deadline, _, self.open = self.jobs.pop(0)
                    try:
                        next(self.open)
                    except StopIteration:
                        self.open = None
                    return True

                def tick(self, tick, budget=2):
                    n = 0
                    while True:
                        urgent = (
                            self.open is not None and self.open_deadline <= tick + 1
                        ) or (
                            self.open is None
                            and self.jobs
                            and self.jobs[0][0] <= tick + 1
                            and self.jobs[0][1] <= tick
                        )
                        if not urgent and n >= budget:
                            break
                        if not self._pull_one(tick):
                            break
                        n += 1

                def drain(self):
                    while self._pull_one(1 << 30):
                        pass

            filler = Filler()

            # ---------------- attention
            class AttnBlock:
                """Heads A=2g, B=2g+1; query half qh (1024 queries).

                scoresT/exp are ACT-paced.  V matmuls run a few kt behind
                (pending FIFO in the driver) so both heads' exp tiles are
                ready together, letting adjacently issued matmuls with
                disjoint array col groups (V: 0-1 vs 2-3) run concurrently
                on the PE.  vt accumulates A in partitions 0-63 and B in
                64-127 of one bank (memset + start=False keeps the
                interleaved accumulation groups from clearing each other).
                The softmax denominators are accumulated OFF the PE: per-kt
                elementwise adds of the exp tiles into a [128,1024] bf16
                accumulator (GpSimd for head A, DVE for head B), reduced
                across partitions by a single 4-matmul quad at block end.
                Normalization runs entirely off the critical path.
                """

                def __init__(self, g, qh):
                    self.g, self.qoff = g, qh * 1024
                    self.vt = [
                        accp.tile([P, 512], F32, tag=f"vt{qb}", name=f"vt{qb}")
                        for qb in range(2)
                    ]
                    self.cs = accp.tile([P, 512], F32, tag="cs")
                    for t in self.vt:
                        nc.vector.memset(t[:], 0.0)
                    nc.vector.memset(self.cs[:], 0.0)
                    self.ets = {}

                def emit_scores_exp(self, kt):
                    g, qoff = self.g, self.qoff
                    # qb-outer, hp-inner: the two heads' matmuls are emitted
                    # adjacently so their row-disjoint array tiles (rows 0-63
                    # vs 64-127) execute concurrently on the PE
                    ps_s = {
                        hp: psp.tile([P, 1024], F32, tag="ps", name=f"ps_s{hp}")
                        for hp in (0, 1)
                    }
                    for qb in range(2):
                        for hp, pb in ((0, 0), (1, 64)):
                            nc.tensor.matmul(
                                ps_s[hp][:, qb * 512 : (qb + 1) * 512],
                                lhsT=KT[g][pb : pb + 64, kt * P : (kt + 1) * P],
                                rhs=QT[g][
                                    pb : pb + 64,
                                    qoff + qb * 512 : qoff + (qb + 1) * 512,
                                ],
                                start=True,
                                stop=True,
                            )
                    for hp in (0, 1):
                        et = expp.tile([P, 1024], BF16, tag="expT", name=f"et{hp}")
                        nc.scalar.activation(
                            et[:], ps_s[hp][:], mybir.ActivationFunctionType.Exp
                        )
                        self.ets[(kt, hp)] = et

                def emit_v(self, kt):
                    g = self.g
                    last = kt == NKT - 1
                    et = self.ets[kt] = {
                        hp: self.ets.pop((kt, hp)) for hp in (0, 1)
                    }
                    for qb in range(2):
                        for hp, pb in ((0, 0), (1, 64)):
                            nc.tensor.matmul(
                                self.vt[qb][pb : pb + 64, :],
                                lhsT=V_st[kt][:, 2 * g + hp, 0:DK],
                                rhs=et[hp][:, qb * 512 : (qb + 1) * 512],
                                start=False,
                                stop=last,
                                skip_group_check=True,
                                tile_position=(0, pb),
                            )

                def emit_cs(self, kt):
                    last = kt == NKT - 1
                    et = self.ets.pop(kt)
                    for hp in (0, 1):
                        for qb in range(2):
                            cp = 64 * hp + 32 * qb
                            nc.tensor.matmul(
                                self.cs[cp : cp + 1, :],
                                lhsT=ones_bf[:],
                                rhs=et[hp][:, qb * 512 : (qb + 1) * 512],
                                start=False,
                                stop=last,
                                skip_group_check=True,
                                tile_position=(0, cp),
                            )
                    if last:
                        self.emit_norm()

                def emit_norm(self):
                    g, qoff = self.g, self.qoff
                    un = [
                        rcp.tile([P, 512], F32, tag=f"un{qb}", name=f"un{qb}")
                        for qb in range(2)
                    ]
                    for qb in range(2):
                        nc.vector.tensor_copy(out=un[qb][:], in_=self.vt[qb][:])
                    cs_sb = rcp.tile([P, 512], F32, tag="cs_sb")
                    nc.vector.tensor_copy(out=cs_sb[:], in_=self.cs[:])
                    zd = dramp.tile([4, 512], F32, name="zd")
                    # zd rows: 0=(A,qb0) 1=(A,qb1) 2=(B,qb0) 3=(B,qb1)
                    nc.sync.dma_start(zd[:], cs_sb[0:128:32, :])
                    # reciprocal on a [128,16] reshape of the 2048 real Z
                    # values (vs [128,512]: DVE reciprocal is ~8 cyc/col)
                    zs = rcp.tile([P, 16], F32, tag="zs")
                    nc.sync.dma_start(
                        zs[:], zd.rearrange("a (b c) -> (a b) c", c=16)
                    )
                    zr = rcp.tile([P, 16], F32, tag="zr")
                    nc.vector.reciprocal(zr[:], zs[:])
                    zd2 = dramp.tile([4, 512], F32, name="zd2")
                    nc.sync.dma_start(
                        zd2.rearrange("a (b c) -> (a b) c", c=16), zr[:]
                    )
                    for qb in range(2):
                        rcb = rcp.tile(
                            [P, 512], F32, tag=f"rcb{qb}", name=f"rcb{qb}"
                        )
                        nc.sync.dma_start(
                            rcb[0:64, :], zd2[qb, None, :].to_broadcast([64, 512])
                        )
                        nc.sync.dma_start(
                            rcb[64:128, :],
                            zd2[qb + 2, None, :].to_broadcast([64, 512]),
                        )
                        for pb in (0, 64):
                            nc.vector.tensor_mul(
                                out=attn[g][
                                    pb : pb + 64,
                                    qoff + qb * 512 : qoff + (qb + 1) * 512,
                                ],
                                in0=un[qb][pb : pb + 64, :],
                                in1=rcb[pb : pb + 64, :],
                            )

            # ---------------- upfront projections: just enough for the first
            # scores matmul (K0 cols 0-511, Q0 cols 0-1023)
            for job in (
                qk_job(KT[0], w_sb["wk"], 0, 0),
                qk_job(QT[0], w_sb["wq"], 0, 0),
                qk_job(QT[0], w_sb["wq"], 0, 1),
            ):
                for _ in job:
                    pass

            # ---------------- filler job schedule (deadlines in global ticks;
            # 1 tick = one (block, kt) step = ~2.2us of exp pacing).
            # Block order: qh-outer -> block index B = qh*4 + g.
            # V_st[st] feeds emit_v_cs at tick st + LAG of block 0.
            # K[g][sc] feeds scores of block g (qh0) at kt=4*sc.
            # Q[g][qh sc] feeds block qh*4+g from its start.
            jobs = []
            # V_st[st] is consumed by the block-0 v_cs pop at pop_tick[st];
            # derive that from the same lag schedule the driver uses so the
            # projection is always fully emitted before its consumer.
            def lag_at(t):
                if t < 8:
                    return 7
                if t < 13:
                    return 14 - t
                return 1 if t % 2 else 2

            pop_tick = {}
            sim_pending, t = [], 0
            while len(pop_tick) < NKT:
                while len(sim_pending) > lag_at(t):
                    pop_tick[sim_pending.pop(0)] = t
                sim_pending.append(t)
                t += 1
            for st in range(NKT):
                jobs.append((pop_tick[st] - 1, max(0, st // 4), v_job(st)))
            for sc in range(1, NSC):
                jobs.append((4 * sc, sc, qk_job(KT[0], w_sb["wk"], 0, sc)))
            def lead(d):
                return d - 8 if d >= 32 else d
            for g in range(1, NG):
                for sc in range(NSC):
                    jobs.append((lead(16 * g + 4 * sc), sc + 1, qk_job(KT[g], w_sb["wk"], g, sc)))
                for sc in range(2):
                    jobs.append((lead(16 * g), sc + 1, qk_job(QT[g], w_sb["wq"], g, sc)))
            for g in range(NG):
                for sc in range(2, NSC):
                    jobs.append((lead(16 * (4 + g)) - 4, sc + 1, qk_job(QT[g], w_sb["wq"], g, sc)))
            # output projection for the first query half: ready once all qh0
            # blocks are normed (~2 ticks after qh0 ends); no hard deadline.
            WO_INJECT = True
            if WO_INJECT:
                for st in range(5):
                    for ob in range(2):
                        jobs.append((1 << 29, 16 * 4 + 3, wo_job(st, ob)))
            jobs.sort(key=lambda j: j[0])
            for d, r, gen in jobs:
                filler.add(d, r, gen)

            # ---------------- attention driver: pending FIFO carries the
            # V/colsum matmuls LAG ticks behind their exps (deep in block 0 to
            # spread the V-projection burst, paired afterwards so the V and
            # colsum matmul groups batch two kt at a time -- every PE weight-
            # geometry switch costs a ~160ns pipeline-drain bubble, so fewer,
            # larger same-shape groups waste less).  Fillers likewise burst on
            # odd ticks only (urgent deadline work still goes out every tick).
            def emit_batch(popped):
                for b, k in popped:
                    b.emit_v(k)
                for b, k in popped:
                    b.emit_cs(k)

            pending = []
            tick = 0
            for qh in range(2):
                for g in range(NG):
                    blk = AttnBlock(g, qh)
                    for kt in range(NKT):
                        blk.emit_scores_exp(kt)
                        lag = lag_at(tick)
                        popped = []
                        while len(pending) > lag:
                            popped.append(pending.pop(0))
                        emit_batch(popped)
                        pending.append((blk, kt))
                        filler.tick(tick, budget=3 if tick % 2 else 0)
                        tick += 1
            while pending:
                emit_batch(pending[:2])
                del pending[:2]
            filler.drain()

            # ---------------- output projection tail.  The held-back qh0
            # chunks (st 5-7) go first: their inputs are long ready, so they
            # keep the PE busy (and the HAM clock-gate warm) while the last
            # block's normalization pipeline drains.
            tail_sts = ([5, 6, 7] if WO_INJECT else list(range(8))) + list(
                range(8, NKT)
            )
            # psp's two slots are free once the last exp has read its scores,
            # so tail chunks double-buffer there (a single fil slot would
            # serialize each chunk behind the previous chunk's DVE copy,
            # idling the PE ~55% and cooling the HAM clock-gate).
            for st in tail_sts:
                for ob in range(2):
                    for _ in wo_job(st, ob, pool=psp, tag="ps"):
                        pass

    _split_sync_waits(nc)
    return nc


_NC = None


def _get_nc():
    global _NC
    if _NC is None:
        _NC = build_nc()
    return _NC


# ---------------------------------------------------------------- host side
def make_in_maps(x, wq, wk, wv, wo):
    x = np.asarray(x, dtype=np.float32)
    wq = np.asarray(wq, dtype=np.float32)
    wk = np.asarray(wk, dtype=np.float32)
    wv = np.asarray(wv, dtype=np.float32)
    wo = np.asarray(wo, dtype=np.float32)
    in_maps = []
    for c in range(N_CORES):
        b, hg = c // 2, c % 2
        sl = slice(hg * DL, (hg + 1) * DL)
        xTc = np.ascontiguousarray(x[b].T).astype(BF16_NP)
        wqTc = np.ascontiguousarray((wq[sl] / 8.0).T).astype(BF16_NP)
        wkTc = np.ascontiguousarray(wk[sl].T).astype(BF16_NP)
        wvTc = np.ascontiguousarray(wv[sl].T).astype(BF16_NP)
        woTc = np.ascontiguousarray(wo[:, sl].T).astype(BF16_NP)
        in_maps.append(
            {"xT": xTc, "wqT": wqTc, "wkT": wkTc, "wvT": wvTc, "woT": woTc}
        )
    return in_maps


def gather(results):
    out = np.zeros((4, S, DM), dtype=np.float32)
    for c in range(N_CORES):
        out[c // 2] += results[c]["out"]
    return out


def kernel(x, wq, wk, wv, wo):
    from concourse.bass_utils import run_bass_kernel_spmd

    nc = _get_nc()
    in_maps = make_in_maps(x, wq, wk, wv, wo)
    res = run_bass_kernel_spmd(nc, in_maps, CORE_IDS)
    return gather(res.results)
